# revision 40
# baseline (speedup 1.0000x reference)
# Trainium2 Bass kernel for nn_DeformableInception (deformable conv x2 -> concat -> 1x1 conv).
#
# Sharding: data-parallel over batch B=8, one sample per NeuronCore (8 cores).
# Weights replicated. No collectives.
#
# Per-core device pipeline (per sample):
#   - x is stored in DRAM as parity-packed row pairs: slot (par, yy, xx) holds
#     image rows (2*yy+par, 2*yy+par+1) x 128ch bf16 (512B). A bilinear 2x2 patch
#     at (yb, xb) is two adjacent slots = ONE contiguous 1KB gather descriptor
#     (>=512B, so no DMA read-modify-write penalty).
#   - per (chunk, branch, tap): SWDGE dma_gather fetches one 1KB patch per output
#     position (positions land on partitions): g[pos, blk, 512] = [v00|v10|v01|v11].
#   - the bilinear blend runs on PE as "diagonal matmuls": for each corner,
#     matmul(out=tp[c, pos], lhsT=g_corner[pos, c], rhs=diag(w_corner)) accumulates
#     the weighted corner into PSUM. The diag tiles (identity * per-position folded
#     corner weight) are built by 4x-mode tensor_scalar on DVE (some on ACT), depend
#     only on host-precomputed weights (not the gather), and are allocated in
#     groups of DIAG_GRP per pool tile to amortize semaphore waits.
#   - tp (f32 PSUM) -> sampT (bf16 SBUF) on ACT, then one PSUM accumulator per
#     chunk takes all 18 taps of both branches: the 1x1 fuse conv and the concat
#     are folded into the per-tap weights on the host (W''_k = W_k @ Wf_br^T),
#     so only a bias-add (ACT activation) and the output DMA remain.
import sys

sys.path.insert(0, "/opt/trn_rl_repo")

import numpy as np
import ml_dtypes

import concourse.bass as bass
import concourse.mybir as mybir
from concourse.tile import TileContext
from concourse.masks import make_identity
from concourse import bacc
from concourse.bass_utils import run_bass_kernel_spmd

bf16 = ml_dtypes.bfloat16

# problem constants (hardcoded per spec)
B = 8
C = 128
H = W = 64
HW = H * W                 # 4096
COUT = 84
K = 3
PAD = 1
KK = K * K                 # 9
NBR = 2                    # two deformable branches
# position chunks; small enough that two PSUM accumulators fit (overlapped
# drain) and the final pipeline drain is short, big enough that gather
# descriptor-prep on Pool stays ahead of the DMA transfers
import os as _osmod
_chunks_env = _osmod.environ.get("KERN_CHUNKS", "2048,2048")
CHUNKS = [int(t) for t in _chunks_env.split(",")]
NCH = len(CHUNKS)
CH_OFF = [sum(CHUNKS[:i]) for i in range(NCH)]
NLISTS = NBR * KK * NCH    # gather lists (one per chunk x branch x tap)
IDXCOLS = HW // 16         # idx cols per (br, tap) across all chunks
WSCOLS = (HW // 128) * 4   # ws cols per (br, tap) across all chunks
NSLOT = 2 * 32 * 64        # 4096 parity-packed patch slots

P = 128
f32 = mybir.dt.float32
bft = mybir.dt.bfloat16
i16 = mybir.dt.int16

import os as _os
# diag builds go to ACT when (i * DIAG_ACT_NUM) % DIAG_ACT_DEN rolls under
DIAG_ACT_NUM = int(_os.environ.get("KERN_DIAG_ACT_NUM", "1"))
DIAG_ACT_DEN = int(_os.environ.get("KERN_DIAG_ACT_DEN", "14"))
# tp->sampT copy engine: 1=DVE, 2=ACT (per-copy round robin list)
COPY_ENGS = _os.environ.get("KERN_COPY_ENGS", "2")
GP_BUFS = int(_os.environ.get("KERN_GP_BUFS", "4"))
TPP_BUFS = int(_os.environ.get("KERN_TPP_BUFS", "2"))
SAMP_BUFS = int(_os.environ.get("KERN_SAMP_BUFS", "3"))
DIAG_BUFS = int(_os.environ.get("KERN_DIAG_BUFS", "3"))
DIAG_GRP = int(_os.environ.get("KERN_DIAG_GRP", "16"))  # diags per pool tile
# PSUM budget: out_ps (f32, CHUNKS[0] wide) banks * bufs + tp 2*TPP_BUFS banks
# must fit 8 banks; double-buffer the accumulator only for chunks <= 1024
BIGP_BUFS = int(_os.environ.get("KERN_BIGP_BUFS", "0"))
if BIGP_BUFS == 0:
    BIGP_BUFS = 2 if CHUNKS[0] <= 1024 else 1

_CACHE = {}


def _host_precompute(x, dm0, dm1, w0, w1, wf, bfv):
    """Numpy precompute: patch-slot gather indices + 2D-folded bilinear corner
    weights, parity-packed x, weight repacks."""
    ky = np.repeat(np.arange(K) - PAD, K).astype(np.float32)
    kx = np.tile(np.arange(K) - PAD, K).astype(np.float32)
    base_y = np.arange(H, dtype=np.float32).reshape(1, 1, H, 1)
    base_x = np.arange(W, dtype=np.float32).reshape(1, 1, 1, W)

    idx_all = np.zeros((B, NBR, KK, HW), np.int16)        # patch slot per (tap,pos)
    w_all = np.zeros((B, NBR, KK, 4, HW), np.float32)     # r0c0,r1c0,r0c1,r1c1

    for br, dm in ((0, dm0), (1, dm1)):
        off = dm.reshape(B, KK, 2, H, W)
        py = off[:, :, 0] + base_y + ky.reshape(1, KK, 1, 1)
        px = off[:, :, 1] + base_x + kx.reshape(1, KK, 1, 1)
        y0 = np.floor(py); x0 = np.floor(px)
        wy1 = py - y0; wx1 = px - x0
        wy0 = 1.0 - wy1; wx0 = 1.0 - wx1
        y0i = y0.astype(np.int64); x0i = x0.astype(np.int64)
        yb = np.clip(y0i, 0, H - 2)
        xb = np.clip(x0i, 0, W - 2)
        slot = (yb & 1) * (32 * 64) + (yb >> 1) * 64 + xb
        idx_all[:, br] = slot.reshape(B, KK, HW).astype(np.int16)
        w4 = np.zeros((2, 2) + py.shape, np.float32)      # [rp, cp, B, KK, H, W]
        for r, wy in ((0, wy0), (1, wy1)):
            yi = y0i + r
            rv = ((yi >= 0) & (yi < H)).astype(np.float32)
            rp = np.clip(yi, 0, H - 1) - yb               # 0 or 1
            for c, wx in ((0, wx0), (1, wx1)):
                xi = x0i + c
                cv = ((xi >= 0) & (xi < W)).astype(np.float32)
                cp = np.clip(xi, 0, W - 1) - xb
                contrib = wy * wx * rv * cv
                for rr in (0, 1):
                    for cc in (0, 1):
                        w4[rr, cc] += np.where((rp == rr) & (cp == cc), contrib, 0.0)
        # corner order matches patch byte layout [v00, v10, v01, v11]
        w_all[:, br, :, 0] = w4[0, 0].reshape(B, KK, HW)
        w_all[:, br, :, 1] = w4[1, 0].reshape(B, KK, HW)
        w_all[:, br, :, 2] = w4[0, 1].reshape(B, KK, HW)
        w_all[:, br, :, 3] = w4[1, 1].reshape(B, KK, HW)

    # xPP [B, NSLOT+2, 2C] bf16: slot (par, yy, xx) = rows (2yy+par, 2yy+par+1)
    xhwc = np.transpose(x, (0, 2, 3, 1))                  # [B, H, W, C]
    xPP = np.zeros((B, 2, 32, 64, 2, C), np.float32)
    for par in (0, 1):
        for rp in (0, 1):
            start = par + rp
            rows = xhwc[:, start::2, :, :]                # [B, n, W, C]
            n = min(rows.shape[1], 32)
            xPP[:, par, :n, :, rp, :] = rows[:, :n]
    xPP = xPP.reshape(B, NSLOT, 2 * C)
    xPPp = np.concatenate([xPP, np.zeros((B, 2, 2 * C), np.float32)], axis=1)
    xPPp = xPPp.astype(bf16)

    # IDX [B, 128, totalcols] int16: lists in consumption order (chunk, br, k),
    # each wrapped (j%16, j//16), replicated across the 8 gpsimd cores'
    # 16-partition groups, then concatenated along cols so one DMA loads it.
    # WS [B, 128, totalwcols] f32 likewise (per block: 4 corner weights).
    idx_cols = []
    ws_cols = []
    for ci in range(NCH):
        off, sz = CH_OFF[ci], CHUNKS[ci]
        for br in range(NBR):
            for k in range(KK):
                seq = idx_all[:, br, k, off:off + sz]     # [B, sz]
                wr = np.transpose(seq.reshape(B, sz // 16, 16), (0, 2, 1))
                wr = np.broadcast_to(wr[:, None, :, :], (B, 8, 16, sz // 16))
                idx_cols.append(wr.reshape(B, P, sz // 16))
                wsl = w_all[:, br, k, :, off:off + sz]    # [B, 4, sz]
                wsl = wsl.reshape(B, 4, sz // 128, P)
                wsl = np.transpose(wsl, (0, 3, 2, 1))     # [B, p, blk, c4]
                ws_cols.append(wsl.reshape(B, P, (sz // 128) * 4))
    IDX = np.ascontiguousarray(np.concatenate(idx_cols, axis=2))
    WS = np.ascontiguousarray(np.concatenate(ws_cols, axis=2), np.float32)

    # W0T [128, NTAPS*COUT] bf16: lhsT per (branch, tap) with the 1x1 fuse conv
    # folded in (W''_k = W_k @ Wf_br^T), device layout
    WFT = [wf[:, :COUT, 0, 0].T, wf[:, COUT:, 0, 0].T]    # [84in, 84out] per br
    W0T = np.zeros((NBR * KK, C, COUT), np.float32)
    for br, w in ((0, w0), (1, w1)):
        for k in range(KK):
            W0T[br * KK + k] = w[:, :, k // K, k % K].T @ WFT[br]
    W0T = np.ascontiguousarray(np.transpose(W0T, (1, 0, 2))).astype(bf16)

    BF = bfv.reshape(COUT, 1).astype(np.float32)
    return xPPp, IDX, WS, W0T, BF


def _build_nc():
    nc = bacc.Bacc()
    xpp_d = nc.declare_dram_parameter("xpp", [NSLOT + 2, 2 * C], bft, isOutput=False)
    idx_d = nc.declare_dram_parameter("idx", [P, NBR * KK * IDXCOLS], i16, isOutput=False)
    ws_d = nc.declare_dram_parameter("ws", [P, NBR * KK * WSCOLS], f32, isOutput=False)
    w0_d = nc.declare_dram_parameter("w0t", [C, NBR * KK * COUT], bft, isOutput=False)
    bf_d = nc.declare_dram_parameter("bfv", [COUT, 1], f32, isOutput=False)
    out_d = nc.declare_dram_parameter("out", [COUT, HW], bft, isOutput=True)

    # patch gather source: elem i = slot i (256 elems), read 512 elems (2 slots)
    src_ap = bass.AP(tensor=xpp_d, offset=0, ap=[[2 * C, NSLOT], [1, 4 * C]])

    copy_engs = [int(t) for t in COPY_ENGS.split(",")]

    with TileContext(nc) as tc:
        with tc.tile_pool(name="const", bufs=1) as const, \
             tc.tile_pool(name="gp", bufs=GP_BUFS) as gp, \
             tc.tile_pool(name="dgp", bufs=DIAG_BUFS) as dgp, \
             tc.tile_pool(name="sampp", bufs=SAMP_BUFS) as sampp, \
             tc.tile_pool(name="op", bufs=2) as op, \
             tc.tile_pool(name="tpp", bufs=TPP_BUFS, space="PSUM") as tpp, \
             tc.tile_pool(name="bigp", bufs=BIGP_BUFS, space="PSUM") as bigp:
            ident = const.tile([P, P], bft)
            make_identity(nc, ident[:])
            TIC = NBR * KK * IDXCOLS
            TWC = NBR * KK * WSCOLS
            # split the idx/ws loads so the first gathers launch ASAP
            NIH = 2 * (CHUNKS[0] // 16)
            NWH = 2 * ((CHUNKS[0] // 128) * 4)
            idx_t = const.tile([P, TIC], i16)
            nc.sync.dma_start(out=idx_t[:, 0:NIH], in_=idx_d[:, 0:NIH])
            ws_t = const.tile([P, TWC], f32)
            nc.sync.dma_start(out=ws_t[:, 0:NWH], in_=ws_d[:, 0:NWH])
            nc.sync.dma_start(out=idx_t[:, NIH:], in_=idx_d[:, NIH:TIC])
            nc.sync.dma_start(out=ws_t[:, NWH:], in_=ws_d[:, NWH:TWC])
            w0_t = const.tile([C, NBR * KK, COUT], bft)
            nc.sync.dma_start(out=w0_t[:], in_=w0_d[:])
            bf_t = const.tile([COUT, 1], f32)
            nc.sync.dma_start(out=bf_t[:], in_=bf_d[:])

            ndiag = 0
            ncopy = 0
            io, wo = 0, 0
            dgrp = None
            for ci in range(NCH):
                off, sz = CH_OFF[ci], CHUNKS[ci]
                nb = sz // 128
                out_ps = bigp.tile([COUT, CHUNKS[0]], f32, tag="big")
                for br in range(NBR):
                    for k in range(KK):
                        t = br * KK + k
                        g = gp.tile([P, CHUNKS[0] // 128, 4 * C], bft, tag="g")
                        nc.gpsimd.dma_gather(
                            out_ap=g[:, 0:nb, :], in_ap=src_ap,
                            idxs_ap=idx_t[:, io:io + sz // 16],
                            num_idxs=sz, num_idxs_reg=sz,
                            elem_size=4 * C, elem_step=2 * C, transpose=False,
                            single_packet=False,
                        )
                        sampT = sampp.tile([C, CHUNKS[0]], bft, tag="sampT")
                        for qh in range((nb + 7) // 8):
                            qnb = min(8, nb - qh * 8)
                            tp = tpp.tile([C, 1024], f32, tag="tp")
                            for jb in range(qnb):
                                b = qh * 8 + jb
                                for c4 in range(4):
                                    gi = ndiag % DIAG_GRP
                                    if gi == 0:
                                        dgrp = dgp.tile([P, DIAG_GRP, P], bft,
                                                        tag="diag")
                                    diag = dgrp[:, gi, :]
                                    sc = ws_t[:, wo + b * 4 + c4:
                                              wo + b * 4 + c4 + 1]
                                    on_act = (ndiag * DIAG_ACT_NUM) \
                                        % DIAG_ACT_DEN < DIAG_ACT_NUM
                                    ndiag += 1
                                    if on_act:
                                        nc.scalar.activation(
                                            out=diag, in_=ident[:],
                                            func=mybir.ActivationFunctionType.Identity,
                                            scale=sc,
                                        )
                                    else:
                                        nc.vector.tensor_scalar(
                                            out=diag, in0=ident[:],
                                            scalar1=sc, scalar2=None,
                                            op0=mybir.AluOpType.mult,
                                        )
                                    nc.tensor.matmul(
                                        out=tp[:, jb * P:(jb + 1) * P],
                                        lhsT=g[:, b, c4 * C:(c4 + 1) * C],
                                        rhs=diag,
                                        start=(c4 == 0), stop=(c4 == 3),
                                    )
                            ce = copy_engs[ncopy % len(copy_engs)]
                            ncopy += 1
                            dst = sampT[:, qh * 1024:qh * 1024 + qnb * P]
                            if ce == 1:
                                nc.vector.tensor_copy(out=dst,
                                                      in_=tp[:, 0:qnb * P])
                            else:
                                nc.scalar.copy(out=dst, in_=tp[:, 0:qnb * P])
                        for cs in range(0, sz, 512):
                            ce_ = min(cs + 512, sz)
                            nc.tensor.matmul(
                                out=out_ps[:, cs:ce_],
                                lhsT=w0_t[:, t, :],
                                rhs=sampT[:, cs:ce_],
                                start=(br == 0 and k == 0),
                                stop=(br == NBR - 1 and k == KK - 1),
                            )
                        io += sz // 16
                        wo += (sz // 128) * 4
                out_sb = op.tile([COUT, CHUNKS[0]], bft, tag="outsb")
                nc.scalar.activation(
                    out=out_sb[:, 0:sz], in_=out_ps[:, 0:sz],
                    func=mybir.ActivationFunctionType.Identity, bias=bf_t[:],
                    scale=1.0,
                )
                nc.sync.dma_start(out=out_d[:, off:off + sz],
                                  in_=out_sb[:, 0:sz])
    nc.finalize()
    return nc


def kernel(x, dm0, dm1, w0, w1, wf, bf):
    x = np.asarray(x, np.float32)
    dm0 = np.asarray(dm0, np.float32)
    dm1 = np.asarray(dm1, np.float32)
    w0 = np.asarray(w0, np.float32)
    w1 = np.asarray(w1, np.float32)
    wf = np.asarray(wf, np.float32)
    bfv = np.asarray(bf, np.float32)

    xPPp, IDX, WS, W0T, BF = _host_precompute(x, dm0, dm1, w0, w1, wf, bfv)

    if "nc" not in _CACHE:
        _CACHE["nc"] = _build_nc()
    nc = _CACHE["nc"]

    in_maps = [
        {
            "xpp": np.ascontiguousarray(xPPp[i]),
            "idx": np.ascontiguousarray(IDX[i].reshape(P, -1)),
            "ws": np.ascontiguousarray(WS[i].reshape(P, -1)),
            "w0t": W0T.reshape(C, -1),
            "bfv": BF,
        }
        for i in range(B)
    ]
    res = run_bass_kernel_spmd(nc, in_maps, core_ids=list(range(B)),
                               **_CACHE.get("run_kwargs", {}))
    _CACHE["last_results"] = res
    out = np.stack([np.asarray(res.results[i]["out"], np.float32)
                    for i in range(B)])
    return out.reshape(B, COUT, H, W)


# revision 44
# speedup vs baseline: 1.0033x; 1.0033x over previous
# Trainium2 Bass kernel for nn_DeformableInception (deformable conv x2 -> concat -> 1x1 conv).
#
# Sharding: data-parallel over batch B=8, one sample per NeuronCore (8 cores).
# Weights replicated. No collectives.
#
# Per-core device pipeline (per sample):
#   - x is stored in DRAM as parity-packed row pairs: slot (par, yy, xx) holds
#     image rows (2*yy+par, 2*yy+par+1) x 128ch bf16 (512B). A bilinear 2x2 patch
#     at (yb, xb) is two adjacent slots = ONE contiguous 1KB gather descriptor
#     (>=512B, so no DMA read-modify-write penalty).
#   - per (chunk, branch, tap): SWDGE dma_gather fetches one 1KB patch per output
#     position (positions land on partitions): g[pos, blk, 512] = [v00|v10|v01|v11].
#   - the bilinear blend runs on PE as "diagonal matmuls": for each corner,
#     matmul(out=tp[c, pos], lhsT=g_corner[pos, c], rhs=diag(w_corner)) accumulates
#     the weighted corner into PSUM. The diag tiles (identity * per-position folded
#     corner weight) are built by 4x-mode tensor_scalar on DVE (some on ACT), depend
#     only on host-precomputed weights (not the gather), and are allocated in
#     groups of DIAG_GRP per pool tile to amortize semaphore waits.
#   - tp (f32 PSUM) -> sampT (bf16 SBUF) on ACT, then one PSUM accumulator per
#     chunk takes all 18 taps of both branches: the 1x1 fuse conv and the concat
#     are folded into the per-tap weights on the host (W''_k = W_k @ Wf_br^T),
#     so only a bias-add (ACT activation) and the output DMA remain.
import sys

sys.path.insert(0, "/opt/trn_rl_repo")

import numpy as np
import ml_dtypes

import concourse.bass as bass
import concourse.mybir as mybir
from concourse.tile import TileContext
from concourse.masks import make_identity
from concourse import bacc
from concourse.bass_utils import run_bass_kernel_spmd

bf16 = ml_dtypes.bfloat16

# problem constants (hardcoded per spec)
B = 8
C = 128
H = W = 64
HW = H * W                 # 4096
COUT = 84
K = 3
PAD = 1
KK = K * K                 # 9
NBR = 2                    # two deformable branches
# position chunks; small enough that two PSUM accumulators fit (overlapped
# drain) and the final pipeline drain is short, big enough that gather
# descriptor-prep on Pool stays ahead of the DMA transfers
import os as _osmod
_chunks_env = _osmod.environ.get("KERN_CHUNKS", "2048,2048")
CHUNKS = [int(t) for t in _chunks_env.split(",")]
NCH = len(CHUNKS)
CH_OFF = [sum(CHUNKS[:i]) for i in range(NCH)]
NLISTS = NBR * KK * NCH    # gather lists (one per chunk x branch x tap)
IDXCOLS = HW // 16         # idx cols per (br, tap) across all chunks
WSCOLS = (HW // 128) * 4   # ws cols per (br, tap) across all chunks
NSLOT = 2 * 32 * 64        # 4096 parity-packed patch slots

P = 128
f32 = mybir.dt.float32
bft = mybir.dt.bfloat16
i16 = mybir.dt.int16

import os as _os
# diag builds go to ACT when (i * DIAG_ACT_NUM) % DIAG_ACT_DEN rolls under
DIAG_ACT_NUM = int(_os.environ.get("KERN_DIAG_ACT_NUM", "1"))
DIAG_ACT_DEN = int(_os.environ.get("KERN_DIAG_ACT_DEN", "14"))
# tp->sampT copy engine: 1=DVE, 2=ACT (per-copy round robin list)
COPY_ENGS = _os.environ.get("KERN_COPY_ENGS", "2")
GP_BUFS = int(_os.environ.get("KERN_GP_BUFS", "4"))
TPP_BUFS = int(_os.environ.get("KERN_TPP_BUFS", "2"))
SAMP_BUFS = int(_os.environ.get("KERN_SAMP_BUFS", "3"))
DIAG_BUFS = int(_os.environ.get("KERN_DIAG_BUFS", "3"))
DIAG_GRP = int(_os.environ.get("KERN_DIAG_GRP", "16"))  # diags per pool tile
# PSUM budget: out_ps (f32, CHUNKS[0] wide) banks * bufs + tp 2*TPP_BUFS banks
# must fit 8 banks; double-buffer the accumulator only for chunks <= 1024
BIGP_BUFS = int(_os.environ.get("KERN_BIGP_BUFS", "0"))
if BIGP_BUFS == 0:
    BIGP_BUFS = 2 if CHUNKS[0] <= 1024 else 1

_CACHE = {}


def _host_precompute(x, dm0, dm1, w0, w1, wf, bfv):
    """Numpy precompute: patch-slot gather indices + 2D-folded bilinear corner
    weights, parity-packed x, weight repacks."""
    ky = np.repeat(np.arange(K) - PAD, K).astype(np.float32)
    kx = np.tile(np.arange(K) - PAD, K).astype(np.float32)
    base_y = np.arange(H, dtype=np.float32).reshape(1, 1, H, 1)
    base_x = np.arange(W, dtype=np.float32).reshape(1, 1, 1, W)

    idx_all = np.zeros((B, NBR, KK, HW), np.int16)        # patch slot per (tap,pos)
    w_all = np.zeros((B, NBR, KK, 4, HW), np.float32)     # r0c0,r1c0,r0c1,r1c1

    for br, dm in ((0, dm0), (1, dm1)):
        off = dm.reshape(B, KK, 2, H, W)
        py = off[:, :, 0] + base_y + ky.reshape(1, KK, 1, 1)
        px = off[:, :, 1] + base_x + kx.reshape(1, KK, 1, 1)
        y0 = np.floor(py); x0 = np.floor(px)
        wy1 = py - y0; wx1 = px - x0
        wy0 = 1.0 - wy1; wx0 = 1.0 - wx1
        y0i = y0.astype(np.int64); x0i = x0.astype(np.int64)
        yb = np.clip(y0i, 0, H - 2)
        xb = np.clip(x0i, 0, W - 2)
        slot = (yb & 1) * (32 * 64) + (yb >> 1) * 64 + xb
        idx_all[:, br] = slot.reshape(B, KK, HW).astype(np.int16)
        w4 = np.zeros((2, 2) + py.shape, np.float32)      # [rp, cp, B, KK, H, W]
        for r, wy in ((0, wy0), (1, wy1)):
            yi = y0i + r
            rv = ((yi >= 0) & (yi < H)).astype(np.float32)
            rp = np.clip(yi, 0, H - 1) - yb               # 0 or 1
            for c, wx in ((0, wx0), (1, wx1)):
                xi = x0i + c
                cv = ((xi >= 0) & (xi < W)).astype(np.float32)
                cp = np.clip(xi, 0, W - 1) - xb
                contrib = wy * wx * rv * cv
                for rr in (0, 1):
                    for cc in (0, 1):
                        w4[rr, cc] += np.where((rp == rr) & (cp == cc), contrib, 0.0)
        # corner order matches patch byte layout [v00, v10, v01, v11]
        w_all[:, br, :, 0] = w4[0, 0].reshape(B, KK, HW)
        w_all[:, br, :, 1] = w4[1, 0].reshape(B, KK, HW)
        w_all[:, br, :, 2] = w4[0, 1].reshape(B, KK, HW)
        w_all[:, br, :, 3] = w4[1, 1].reshape(B, KK, HW)

    # xPP [B, NSLOT+2, 2C] bf16: slot (par, yy, xx) = rows (2yy+par, 2yy+par+1)
    xhwc = np.transpose(x, (0, 2, 3, 1))                  # [B, H, W, C]
    xPP = np.zeros((B, 2, 32, 64, 2, C), np.float32)
    for par in (0, 1):
        for rp in (0, 1):
            start = par + rp
            rows = xhwc[:, start::2, :, :]                # [B, n, W, C]
            n = min(rows.shape[1], 32)
            xPP[:, par, :n, :, rp, :] = rows[:, :n]
    xPP = xPP.reshape(B, NSLOT, 2 * C)
    xPPp = np.concatenate([xPP, np.zeros((B, 2, 2 * C), np.float32)], axis=1)
    xPPp = xPPp.astype(bf16)

    # IDX [B, 128, totalcols] int16: lists in consumption order (chunk, br, k),
    # each wrapped (j%16, j//16), replicated across the 8 gpsimd cores'
    # 16-partition groups, then concatenated along cols so one DMA loads it.
    # WS [B, 128, totalwcols] f32 likewise (per block: 4 corner weights).
    idx_cols = []
    ws_cols = []
    for ci in range(NCH):
        off, sz = CH_OFF[ci], CHUNKS[ci]
        for br in range(NBR):
            for k in range(KK):
                seq = idx_all[:, br, k, off:off + sz]     # [B, sz]
                wr = np.transpose(seq.reshape(B, sz // 16, 16), (0, 2, 1))
                wr = np.broadcast_to(wr[:, None, :, :], (B, 8, 16, sz // 16))
                idx_cols.append(wr.reshape(B, P, sz // 16))
                wsl = w_all[:, br, k, :, off:off + sz]    # [B, 4, sz]
                wsl = wsl.reshape(B, 4, sz // 128, P)
                wsl = np.transpose(wsl, (0, 3, 2, 1))     # [B, p, blk, c4]
                ws_cols.append(wsl.reshape(B, P, (sz // 128) * 4))
    IDX = np.ascontiguousarray(np.concatenate(idx_cols, axis=2))
    WS = np.ascontiguousarray(np.concatenate(ws_cols, axis=2), np.float32)

    # W0T [128, NTAPS*COUT] bf16: lhsT per (branch, tap) with the 1x1 fuse conv
    # folded in (W''_k = W_k @ Wf_br^T), device layout
    WFT = [wf[:, :COUT, 0, 0].T, wf[:, COUT:, 0, 0].T]    # [84in, 84out] per br
    W0T = np.zeros((NBR * KK, C, COUT), np.float32)
    for br, w in ((0, w0), (1, w1)):
        for k in range(KK):
            W0T[br * KK + k] = w[:, :, k // K, k % K].T @ WFT[br]
    W0T = np.ascontiguousarray(np.transpose(W0T, (1, 0, 2))).astype(bf16)

    BF = bfv.reshape(COUT, 1).astype(np.float32)
    return xPPp, IDX, WS, W0T, BF


def _build_nc():
    nc = bacc.Bacc()
    xpp_d = nc.declare_dram_parameter("xpp", [NSLOT + 2, 2 * C], bft, isOutput=False)
    idx_d = nc.declare_dram_parameter("idx", [P, NBR * KK * IDXCOLS], i16, isOutput=False)
    ws_d = nc.declare_dram_parameter("ws", [P, NBR * KK * WSCOLS], f32, isOutput=False)
    w0_d = nc.declare_dram_parameter("w0t", [C, NBR * KK * COUT], bft, isOutput=False)
    bf_d = nc.declare_dram_parameter("bfv", [COUT, 1], f32, isOutput=False)
    out_d = nc.declare_dram_parameter("out", [COUT, HW], bft, isOutput=True)

    # patch gather source: elem i = slot i (256 elems), read 512 elems (2 slots)
    src_ap = bass.AP(tensor=xpp_d, offset=0, ap=[[2 * C, NSLOT], [1, 4 * C]])

    copy_engs = [int(t) for t in COPY_ENGS.split(",")]

    with TileContext(nc) as tc:
        with tc.tile_pool(name="const", bufs=1) as const, \
             tc.tile_pool(name="gp", bufs=GP_BUFS) as gp, \
             tc.tile_pool(name="dgp", bufs=DIAG_BUFS) as dgp, \
             tc.tile_pool(name="sampp", bufs=SAMP_BUFS) as sampp, \
             tc.tile_pool(name="op", bufs=2) as op, \
             tc.tile_pool(name="tpp", bufs=TPP_BUFS, space="PSUM") as tpp, \
             tc.tile_pool(name="bigp", bufs=BIGP_BUFS, space="PSUM") as bigp:
            ident = const.tile([P, P], bft)
            make_identity(nc, ident[:])
            TIC = NBR * KK * IDXCOLS
            TWC = NBR * KK * WSCOLS
            # split the idx/ws loads so the first gathers launch ASAP
            NIH = 2 * (CHUNKS[0] // 16)
            NWH = 2 * ((CHUNKS[0] // 128) * 4)
            idx_t = const.tile([P, TIC], i16)
            nc.sync.dma_start(out=idx_t[:, 0:NIH], in_=idx_d[:, 0:NIH])
            ws_t = const.tile([P, TWC], f32)
            nc.sync.dma_start(out=ws_t[:, 0:NWH], in_=ws_d[:, 0:NWH])
            nc.sync.dma_start(out=idx_t[:, NIH:], in_=idx_d[:, NIH:TIC])
            nc.sync.dma_start(out=ws_t[:, NWH:], in_=ws_d[:, NWH:TWC])
            w0_t = const.tile([C, NBR * KK, COUT], bft)
            nc.sync.dma_start(out=w0_t[:], in_=w0_d[:])
            bf_t = const.tile([COUT, 1], f32)
            nc.sync.dma_start(out=bf_t[:], in_=bf_d[:])

            ndiag = 0
            ncopy = 0
            io, wo = 0, 0
            dgrp = None
            for ci in range(NCH):
                off, sz = CH_OFF[ci], CHUNKS[ci]
                nb = sz // 128
                out_ps = bigp.tile([COUT, CHUNKS[0]], f32, tag="big")
                for br in range(NBR):
                    for k in range(KK):
                        t = br * KK + k
                        last = (ci == NCH - 1 and br == NBR - 1
                                and k == KK - 1 and sz >= 1024)
                        g = gp.tile([P, CHUNKS[0] // 128, 4 * C], bft, tag="g")
                        if last:
                            # split the final gather so its first half can be
                            # processed while the second half transfers
                            hsz = sz // 2
                            for gh in range(2):
                                nc.gpsimd.dma_gather(
                                    out_ap=g[:, gh * (hsz // 128):
                                             gh * (hsz // 128) + hsz // 128, :],
                                    in_ap=src_ap,
                                    idxs_ap=idx_t[:, io + gh * (hsz // 16):
                                                  io + (gh + 1) * (hsz // 16)],
                                    num_idxs=hsz, num_idxs_reg=hsz,
                                    elem_size=4 * C, elem_step=2 * C,
                                    transpose=False, single_packet=False,
                                )
                        else:
                            nc.gpsimd.dma_gather(
                                out_ap=g[:, 0:nb, :], in_ap=src_ap,
                                idxs_ap=idx_t[:, io:io + sz // 16],
                                num_idxs=sz, num_idxs_reg=sz,
                                elem_size=4 * C, elem_step=2 * C, transpose=False,
                                single_packet=False,
                            )
                        sampT = sampp.tile([C, CHUNKS[0]], bft, tag="sampT")
                        stripb = 4 if last else 8
                        for qh in range((nb + stripb - 1) // stripb):
                            qnb = min(stripb, nb - qh * stripb)
                            tp = tpp.tile([C, 1024], f32, tag="tp")
                            for jb in range(qnb):
                                b = qh * stripb + jb
                                for c4 in range(4):
                                    gi = ndiag % DIAG_GRP
                                    if gi == 0:
                                        dgrp = dgp.tile([P, DIAG_GRP, P], bft,
                                                        tag="diag")
                                    diag = dgrp[:, gi, :]
                                    sc = ws_t[:, wo + b * 4 + c4:
                                              wo + b * 4 + c4 + 1]
                                    on_act = (ndiag * DIAG_ACT_NUM) \
                                        % DIAG_ACT_DEN < DIAG_ACT_NUM
                                    ndiag += 1
                                    if on_act:
                                        nc.scalar.activation(
                                            out=diag, in_=ident[:],
                                            func=mybir.ActivationFunctionType.Identity,
                                            scale=sc,
                                        )
                                    else:
                                        nc.vector.tensor_scalar(
                                            out=diag, in0=ident[:],
                                            scalar1=sc, scalar2=None,
                                            op0=mybir.AluOpType.mult,
                                        )
                                    nc.tensor.matmul(
                                        out=tp[:, jb * P:(jb + 1) * P],
                                        lhsT=g[:, b, c4 * C:(c4 + 1) * C],
                                        rhs=diag,
                                        start=(c4 == 0), stop=(c4 == 3),
                                    )
                            ce = copy_engs[ncopy % len(copy_engs)]
                            ncopy += 1
                            qoff = qh * stripb * P
                            dst = sampT[:, qoff:qoff + qnb * P]
                            if ce == 1:
                                nc.vector.tensor_copy(out=dst,
                                                      in_=tp[:, 0:qnb * P])
                            else:
                                nc.scalar.copy(out=dst, in_=tp[:, 0:qnb * P])
                            if last:
                                # deform per 512-strip so the drain pipelines
                                nc.tensor.matmul(
                                    out=out_ps[:, qoff:qoff + qnb * P],
                                    lhsT=w0_t[:, t, :],
                                    rhs=sampT[:, qoff:qoff + qnb * P],
                                    start=False, stop=True,
                                )
                        if not last:
                            for cs in range(0, sz, 512):
                                ce_ = min(cs + 512, sz)
                                nc.tensor.matmul(
                                    out=out_ps[:, cs:ce_],
                                    lhsT=w0_t[:, t, :],
                                    rhs=sampT[:, cs:ce_],
                                    start=(br == 0 and k == 0),
                                    stop=(br == NBR - 1 and k == KK - 1),
                                )
                        io += sz // 16
                        wo += (sz // 128) * 4
                out_sb = op.tile([COUT, CHUNKS[0]], bft, tag="outsb")
                nc.scalar.activation(
                    out=out_sb[:, 0:sz], in_=out_ps[:, 0:sz],
                    func=mybir.ActivationFunctionType.Identity, bias=bf_t[:],
                    scale=1.0,
                )
                nc.sync.dma_start(out=out_d[:, off:off + sz],
                                  in_=out_sb[:, 0:sz])
    nc.finalize()
    return nc


def kernel(x, dm0, dm1, w0, w1, wf, bf):
    x = np.asarray(x, np.float32)
    dm0 = np.asarray(dm0, np.float32)
    dm1 = np.asarray(dm1, np.float32)
    w0 = np.asarray(w0, np.float32)
    w1 = np.asarray(w1, np.float32)
    wf = np.asarray(wf, np.float32)
    bfv = np.asarray(bf, np.float32)

    xPPp, IDX, WS, W0T, BF = _host_precompute(x, dm0, dm1, w0, w1, wf, bfv)

    if "nc" not in _CACHE:
        _CACHE["nc"] = _build_nc()
    nc = _CACHE["nc"]

    in_maps = [
        {
            "xpp": np.ascontiguousarray(xPPp[i]),
            "idx": np.ascontiguousarray(IDX[i].reshape(P, -1)),
            "ws": np.ascontiguousarray(WS[i].reshape(P, -1)),
            "w0t": W0T.reshape(C, -1),
            "bfv": BF,
        }
        for i in range(B)
    ]
    res = run_bass_kernel_spmd(nc, in_maps, core_ids=list(range(B)),
                               **_CACHE.get("run_kwargs", {}))
    _CACHE["last_results"] = res
    out = np.stack([np.asarray(res.results[i]["out"], np.float32)
                    for i in range(B)])
    return out.reshape(B, COUT, H, W)


# revision 47
# speedup vs baseline: 1.0064x; 1.0030x over previous
# Trainium2 Bass kernel for nn_DeformableInception (deformable conv x2 -> concat -> 1x1 conv).
#
# Sharding: data-parallel over batch B=8, one sample per NeuronCore (8 cores).
# Weights replicated. No collectives.
#
# Per-core device pipeline (per sample):
#   - x is stored in DRAM as parity-packed row pairs: slot (par, yy, xx) holds
#     image rows (2*yy+par, 2*yy+par+1) x 128ch bf16 (512B). A bilinear 2x2 patch
#     at (yb, xb) is two adjacent slots = ONE contiguous 1KB gather descriptor
#     (>=512B, so no DMA read-modify-write penalty).
#   - per (chunk, branch, tap): SWDGE dma_gather fetches one 1KB patch per output
#     position (positions land on partitions): g[pos, blk, 512] = [v00|v10|v01|v11].
#   - the bilinear blend runs on PE as "diagonal matmuls": for each corner,
#     matmul(out=tp[c, pos], lhsT=g_corner[pos, c], rhs=diag(w_corner)) accumulates
#     the weighted corner into PSUM. The diag tiles (identity * per-position folded
#     corner weight) are built by 4x-mode tensor_scalar on DVE (some on ACT), depend
#     only on host-precomputed weights (not the gather), and are allocated in
#     groups of DIAG_GRP per pool tile to amortize semaphore waits.
#   - tp (f32 PSUM) -> sampT (bf16 SBUF) on ACT, then one PSUM accumulator per
#     chunk takes all 18 taps of both branches: the 1x1 fuse conv and the concat
#     are folded into the per-tap weights on the host (W''_k = W_k @ Wf_br^T),
#     so only a bias-add (ACT activation) and the output DMA remain.
import sys

sys.path.insert(0, "/opt/trn_rl_repo")

import numpy as np
import ml_dtypes

import concourse.bass as bass
import concourse.mybir as mybir
from concourse.tile import TileContext
from concourse.masks import make_identity
from concourse import bacc
from concourse.bass_utils import run_bass_kernel_spmd

bf16 = ml_dtypes.bfloat16

# problem constants (hardcoded per spec)
B = 8
C = 128
H = W = 64
HW = H * W                 # 4096
COUT = 84
K = 3
PAD = 1
KK = K * K                 # 9
NBR = 2                    # two deformable branches
# position chunks; small enough that two PSUM accumulators fit (overlapped
# drain) and the final pipeline drain is short, big enough that gather
# descriptor-prep on Pool stays ahead of the DMA transfers
import os as _osmod
_chunks_env = _osmod.environ.get("KERN_CHUNKS", "2048,2048")
CHUNKS = [int(t) for t in _chunks_env.split(",")]
NCH = len(CHUNKS)
CH_OFF = [sum(CHUNKS[:i]) for i in range(NCH)]
NLISTS = NBR * KK * NCH    # gather lists (one per chunk x branch x tap)
IDXCOLS = HW // 16         # idx cols per (br, tap) across all chunks
WSCOLS = (HW // 128) * 4   # ws cols per (br, tap) across all chunks
NSLOT = 2 * 32 * 64        # 4096 parity-packed patch slots

P = 128
f32 = mybir.dt.float32
bft = mybir.dt.bfloat16
i16 = mybir.dt.int16

import os as _os
# diag builds go to ACT when (i * DIAG_ACT_NUM) % DIAG_ACT_DEN rolls under
DIAG_ACT_NUM = int(_os.environ.get("KERN_DIAG_ACT_NUM", "1"))
DIAG_ACT_DEN = int(_os.environ.get("KERN_DIAG_ACT_DEN", "14"))
# tp->sampT copy engine: 1=DVE, 2=ACT (per-copy round robin list)
COPY_ENGS = _os.environ.get("KERN_COPY_ENGS", "2")
GP_BUFS = int(_os.environ.get("KERN_GP_BUFS", "4"))
TPP_BUFS = int(_os.environ.get("KERN_TPP_BUFS", "2"))
SAMP_BUFS = int(_os.environ.get("KERN_SAMP_BUFS", "3"))
DIAG_BUFS = int(_os.environ.get("KERN_DIAG_BUFS", "3"))
DIAG_GRP = int(_os.environ.get("KERN_DIAG_GRP", "16"))  # diags per pool tile
# PSUM budget: out_ps (f32, CHUNKS[0] wide) banks * bufs + tp 2*TPP_BUFS banks
# must fit 8 banks; double-buffer the accumulator only for chunks <= 1024
BIGP_BUFS = int(_os.environ.get("KERN_BIGP_BUFS", "0"))
if BIGP_BUFS == 0:
    BIGP_BUFS = 2 if CHUNKS[0] <= 1024 else 1

_CACHE = {}


def _host_precompute(x, dm0, dm1, w0, w1, wf, bfv):
    """Numpy precompute: patch-slot gather indices + 2D-folded bilinear corner
    weights, parity-packed x, weight repacks."""
    ky = np.repeat(np.arange(K) - PAD, K).astype(np.float32)
    kx = np.tile(np.arange(K) - PAD, K).astype(np.float32)
    base_y = np.arange(H, dtype=np.float32).reshape(1, 1, H, 1)
    base_x = np.arange(W, dtype=np.float32).reshape(1, 1, 1, W)

    idx_all = np.zeros((B, NBR, KK, HW), np.int16)        # patch slot per (tap,pos)
    w_all = np.zeros((B, NBR, KK, 4, HW), np.float32)     # r0c0,r1c0,r0c1,r1c1

    for br, dm in ((0, dm0), (1, dm1)):
        off = dm.reshape(B, KK, 2, H, W)
        py = off[:, :, 0] + base_y + ky.reshape(1, KK, 1, 1)
        px = off[:, :, 1] + base_x + kx.reshape(1, KK, 1, 1)
        y0 = np.floor(py); x0 = np.floor(px)
        wy1 = py - y0; wx1 = px - x0
        wy0 = 1.0 - wy1; wx0 = 1.0 - wx1
        y0i = y0.astype(np.int64); x0i = x0.astype(np.int64)
        yb = np.clip(y0i, 0, H - 2)
        xb = np.clip(x0i, 0, W - 2)
        slot = (yb & 1) * (32 * 64) + (yb >> 1) * 64 + xb
        idx_all[:, br] = slot.reshape(B, KK, HW).astype(np.int16)
        w4 = np.zeros((2, 2) + py.shape, np.float32)      # [rp, cp, B, KK, H, W]
        for r, wy in ((0, wy0), (1, wy1)):
            yi = y0i + r
            rv = ((yi >= 0) & (yi < H)).astype(np.float32)
            rp = np.clip(yi, 0, H - 1) - yb               # 0 or 1
            for c, wx in ((0, wx0), (1, wx1)):
                xi = x0i + c
                cv = ((xi >= 0) & (xi < W)).astype(np.float32)
                cp = np.clip(xi, 0, W - 1) - xb
                contrib = wy * wx * rv * cv
                for rr in (0, 1):
                    for cc in (0, 1):
                        w4[rr, cc] += np.where((rp == rr) & (cp == cc), contrib, 0.0)
        # corner order matches patch byte layout [v00, v10, v01, v11]
        w_all[:, br, :, 0] = w4[0, 0].reshape(B, KK, HW)
        w_all[:, br, :, 1] = w4[1, 0].reshape(B, KK, HW)
        w_all[:, br, :, 2] = w4[0, 1].reshape(B, KK, HW)
        w_all[:, br, :, 3] = w4[1, 1].reshape(B, KK, HW)

    # xPP [B, NSLOT+2, 2C] bf16: slot (par, yy, xx) = rows (2yy+par, 2yy+par+1)
    xhwc = np.transpose(x, (0, 2, 3, 1))                  # [B, H, W, C]
    xPP = np.zeros((B, 2, 32, 64, 2, C), np.float32)
    for par in (0, 1):
        for rp in (0, 1):
            start = par + rp
            rows = xhwc[:, start::2, :, :]                # [B, n, W, C]
            n = min(rows.shape[1], 32)
            xPP[:, par, :n, :, rp, :] = rows[:, :n]
    xPP = xPP.reshape(B, NSLOT, 2 * C)
    xPPp = np.concatenate([xPP, np.zeros((B, 2, 2 * C), np.float32)], axis=1)
    xPPp = xPPp.astype(bf16)

    # IDX [B, 128, totalcols] int16: lists in consumption order (chunk, br, k),
    # each wrapped (j%16, j//16), replicated across the 8 gpsimd cores'
    # 16-partition groups, then concatenated along cols so one DMA loads it.
    # WS [B, 128, totalwcols] f32 likewise (per block: 4 corner weights).
    idx_cols = []
    ws_cols = []
    for ci in range(NCH):
        off, sz = CH_OFF[ci], CHUNKS[ci]
        for br in range(NBR):
            for k in range(KK):
                seq = idx_all[:, br, k, off:off + sz]     # [B, sz]
                wr = np.transpose(seq.reshape(B, sz // 16, 16), (0, 2, 1))
                wr = np.broadcast_to(wr[:, None, :, :], (B, 8, 16, sz // 16))
                idx_cols.append(wr.reshape(B, P, sz // 16))
                wsl = w_all[:, br, k, :, off:off + sz]    # [B, 4, sz]
                wsl = wsl.reshape(B, 4, sz // 128, P)
                wsl = np.transpose(wsl, (0, 3, 2, 1))     # [B, p, blk, c4]
                ws_cols.append(wsl.reshape(B, P, (sz // 128) * 4))
    IDX = np.ascontiguousarray(np.concatenate(idx_cols, axis=2))
    WS = np.ascontiguousarray(np.concatenate(ws_cols, axis=2), np.float32)

    # W0T [128, NTAPS*COUT] bf16: lhsT per (branch, tap) with the 1x1 fuse conv
    # folded in (W''_k = W_k @ Wf_br^T), device layout
    WFT = [wf[:, :COUT, 0, 0].T, wf[:, COUT:, 0, 0].T]    # [84in, 84out] per br
    W0T = np.zeros((NBR * KK, C, COUT), np.float32)
    for br, w in ((0, w0), (1, w1)):
        for k in range(KK):
            W0T[br * KK + k] = w[:, :, k // K, k % K].T @ WFT[br]
    W0T = np.ascontiguousarray(np.transpose(W0T, (1, 0, 2))).astype(bf16)

    BF = bfv.reshape(COUT, 1).astype(np.float32)
    return xPPp, IDX, WS, W0T, BF


def _build_nc():
    nc = bacc.Bacc()
    xpp_d = nc.declare_dram_parameter("xpp", [NSLOT + 2, 2 * C], bft, isOutput=False)
    idx_d = nc.declare_dram_parameter("idx", [P, NBR * KK * IDXCOLS], i16, isOutput=False)
    ws_d = nc.declare_dram_parameter("ws", [P, NBR * KK * WSCOLS], f32, isOutput=False)
    w0_d = nc.declare_dram_parameter("w0t", [C, NBR * KK * COUT], bft, isOutput=False)
    bf_d = nc.declare_dram_parameter("bfv", [COUT, 1], f32, isOutput=False)
    out_d = nc.declare_dram_parameter("out", [COUT, HW], bft, isOutput=True)

    # patch gather source: elem i = slot i (256 elems), read 512 elems (2 slots)
    src_ap = bass.AP(tensor=xpp_d, offset=0, ap=[[2 * C, NSLOT], [1, 4 * C]])

    copy_engs = [int(t) for t in COPY_ENGS.split(",")]

    with TileContext(nc) as tc:
        with tc.tile_pool(name="const", bufs=1) as const, \
             tc.tile_pool(name="gp", bufs=GP_BUFS) as gp, \
             tc.tile_pool(name="dgp", bufs=DIAG_BUFS) as dgp, \
             tc.tile_pool(name="sampp", bufs=SAMP_BUFS) as sampp, \
             tc.tile_pool(name="op", bufs=2) as op, \
             tc.tile_pool(name="tpp", bufs=TPP_BUFS, space="PSUM") as tpp, \
             tc.tile_pool(name="bigp", bufs=BIGP_BUFS, space="PSUM") as bigp:
            ident = const.tile([P, P], bft)
            make_identity(nc, ident[:])
            TIC = NBR * KK * IDXCOLS
            TWC = NBR * KK * WSCOLS
            # split the idx/ws loads so the first gathers launch ASAP
            NIH = 2 * (CHUNKS[0] // 16)
            NWH = 2 * ((CHUNKS[0] // 128) * 4)
            idx_t = const.tile([P, TIC], i16)
            nc.sync.dma_start(out=idx_t[:, 0:NIH], in_=idx_d[:, 0:NIH])
            ws_t = const.tile([P, TWC], f32)
            nc.sync.dma_start(out=ws_t[:, 0:NWH], in_=ws_d[:, 0:NWH])
            nc.sync.dma_start(out=idx_t[:, NIH:], in_=idx_d[:, NIH:TIC])
            nc.sync.dma_start(out=ws_t[:, NWH:], in_=ws_d[:, NWH:TWC])
            w0_t = const.tile([C, NBR * KK, COUT], bft)
            nc.sync.dma_start(out=w0_t[:], in_=w0_d[:])
            bf_t = const.tile([COUT, 1], f32)
            nc.sync.dma_start(out=bf_t[:], in_=bf_d[:])

            ndiag = 0
            ncopy = 0
            io, wo = 0, 0
            dgrp = None
            for ci in range(NCH):
                off, sz = CH_OFF[ci], CHUNKS[ci]
                nb = sz // 128
                out_ps = bigp.tile([COUT, CHUNKS[0]], f32, tag="big")
                for br in range(NBR):
                    for k in range(KK):
                        t = br * KK + k
                        last = (ci == NCH - 1 and br == NBR - 1
                                and k == KK - 1 and sz >= 1024)
                        g = gp.tile([P, CHUNKS[0] // 128, 4 * C], bft, tag="g")
                        if last:
                            # split the final gather so its first half can be
                            # processed while the second half transfers
                            hsz = sz // 2
                            for gh in range(2):
                                nc.gpsimd.dma_gather(
                                    out_ap=g[:, gh * (hsz // 128):
                                             gh * (hsz // 128) + hsz // 128, :],
                                    in_ap=src_ap,
                                    idxs_ap=idx_t[:, io + gh * (hsz // 16):
                                                  io + (gh + 1) * (hsz // 16)],
                                    num_idxs=hsz, num_idxs_reg=hsz,
                                    elem_size=4 * C, elem_step=2 * C,
                                    transpose=False, single_packet=False,
                                )
                        else:
                            nc.gpsimd.dma_gather(
                                out_ap=g[:, 0:nb, :], in_ap=src_ap,
                                idxs_ap=idx_t[:, io:io + sz // 16],
                                num_idxs=sz, num_idxs_reg=sz,
                                elem_size=4 * C, elem_step=2 * C, transpose=False,
                                single_packet=False,
                            )
                        sampT = sampp.tile([C, CHUNKS[0]], bft, tag="sampT")
                        stripb = 4 if last else 8
                        for qh in range((nb + stripb - 1) // stripb):
                            qnb = min(stripb, nb - qh * stripb)
                            tp = tpp.tile([C, 1024], f32, tag="tp")
                            for jb in range(qnb):
                                b = qh * stripb + jb
                                for c4 in range(4):
                                    gi = ndiag % DIAG_GRP
                                    if gi == 0:
                                        dgrp = dgp.tile([P, DIAG_GRP, P], bft,
                                                        tag="diag")
                                    diag = dgrp[:, gi, :]
                                    sc = ws_t[:, wo + b * 4 + c4:
                                              wo + b * 4 + c4 + 1]
                                    on_act = (ndiag * DIAG_ACT_NUM) \
                                        % DIAG_ACT_DEN < DIAG_ACT_NUM
                                    ndiag += 1
                                    if on_act:
                                        nc.scalar.activation(
                                            out=diag, in_=ident[:],
                                            func=mybir.ActivationFunctionType.Identity,
                                            scale=sc,
                                        )
                                    else:
                                        nc.vector.tensor_scalar(
                                            out=diag, in0=ident[:],
                                            scalar1=sc, scalar2=None,
                                            op0=mybir.AluOpType.mult,
                                        )
                                    nc.tensor.matmul(
                                        out=tp[:, jb * P:(jb + 1) * P],
                                        lhsT=g[:, b, c4 * C:(c4 + 1) * C],
                                        rhs=diag,
                                        start=(c4 == 0), stop=(c4 == 3),
                                    )
                            ce = copy_engs[ncopy % len(copy_engs)]
                            ncopy += 1
                            qoff = qh * stripb * P
                            dst = sampT[:, qoff:qoff + qnb * P]
                            if ce == 1:
                                nc.vector.tensor_copy(out=dst,
                                                      in_=tp[:, 0:qnb * P])
                            else:
                                nc.scalar.copy(out=dst, in_=tp[:, 0:qnb * P])
                            if last:
                                # deform per 512-strip so the drain pipelines
                                nc.tensor.matmul(
                                    out=out_ps[:, qoff:qoff + qnb * P],
                                    lhsT=w0_t[:, t, :],
                                    rhs=sampT[:, qoff:qoff + qnb * P],
                                    start=False, stop=True,
                                )
                                if qh == sz // 1024 - 1:
                                    # first half fully accumulated: drain it
                                    # while the remaining strips process
                                    out_sb = op.tile([COUT, CHUNKS[0]], bft,
                                                     tag="outsb")
                                    nc.scalar.activation(
                                        out=out_sb[:, 0:sz // 2],
                                        in_=out_ps[:, 0:sz // 2],
                                        func=mybir.ActivationFunctionType.Identity,
                                        bias=bf_t[:], scale=1.0,
                                    )
                                    nc.sync.dma_start(
                                        out=out_d[:, off:off + sz // 2],
                                        in_=out_sb[:, 0:sz // 2])
                        if not last:
                            for cs in range(0, sz, 512):
                                ce_ = min(cs + 512, sz)
                                nc.tensor.matmul(
                                    out=out_ps[:, cs:ce_],
                                    lhsT=w0_t[:, t, :],
                                    rhs=sampT[:, cs:ce_],
                                    start=(br == 0 and k == 0),
                                    stop=(br == NBR - 1 and k == KK - 1),
                                )
                        io += sz // 16
                        wo += (sz // 128) * 4
                out_sb = op.tile([COUT, CHUNKS[0]], bft, tag="outsb")
                if ci == NCH - 1 and sz >= 1024:
                    # second half only: the first half drained mid-strip-loop
                    nc.scalar.activation(
                        out=out_sb[:, sz // 2:sz], in_=out_ps[:, sz // 2:sz],
                        func=mybir.ActivationFunctionType.Identity,
                        bias=bf_t[:], scale=1.0,
                    )
                    nc.sync.dma_start(out=out_d[:, off + sz // 2:off + sz],
                                      in_=out_sb[:, sz // 2:sz])
                else:
                    nc.scalar.activation(
                        out=out_sb[:, 0:sz], in_=out_ps[:, 0:sz],
                        func=mybir.ActivationFunctionType.Identity, bias=bf_t[:],
                        scale=1.0,
                    )
                    nc.sync.dma_start(out=out_d[:, off:off + sz],
                                      in_=out_sb[:, 0:sz])
    nc.finalize()
    return nc


def kernel(x, dm0, dm1, w0, w1, wf, bf):
    x = np.asarray(x, np.float32)
    dm0 = np.asarray(dm0, np.float32)
    dm1 = np.asarray(dm1, np.float32)
    w0 = np.asarray(w0, np.float32)
    w1 = np.asarray(w1, np.float32)
    wf = np.asarray(wf, np.float32)
    bfv = np.asarray(bf, np.float32)

    xPPp, IDX, WS, W0T, BF = _host_precompute(x, dm0, dm1, w0, w1, wf, bfv)

    if "nc" not in _CACHE:
        _CACHE["nc"] = _build_nc()
    nc = _CACHE["nc"]

    in_maps = [
        {
            "xpp": np.ascontiguousarray(xPPp[i]),
            "idx": np.ascontiguousarray(IDX[i].reshape(P, -1)),
            "ws": np.ascontiguousarray(WS[i].reshape(P, -1)),
            "w0t": W0T.reshape(C, -1),
            "bfv": BF,
        }
        for i in range(B)
    ]
    res = run_bass_kernel_spmd(nc, in_maps, core_ids=list(range(B)),
                               **_CACHE.get("run_kwargs", {}))
    _CACHE["last_results"] = res
    out = np.stack([np.asarray(res.results[i]["out"], np.float32)
                    for i in range(B)])
    return out.reshape(B, COUT, H, W)


# revision 51
# speedup vs baseline: 1.0073x; 1.0009x over previous
# Trainium2 Bass kernel for nn_DeformableInception (deformable conv x2 -> concat -> 1x1 conv).
#
# Sharding: data-parallel over batch B=8, one sample per NeuronCore (8 cores).
# Weights replicated. No collectives.
#
# Per-core device pipeline (per sample):
#   - x is stored in DRAM as parity-packed row pairs: slot (par, yy, xx) holds
#     image rows (2*yy+par, 2*yy+par+1) x 128ch bf16 (512B). A bilinear 2x2 patch
#     at (yb, xb) is two adjacent slots = ONE contiguous 1KB gather descriptor
#     (>=512B, so no DMA read-modify-write penalty).
#   - per (chunk, branch, tap): SWDGE dma_gather fetches one 1KB patch per output
#     position (positions land on partitions): g[pos, blk, 512] = [v00|v10|v01|v11].
#   - the bilinear blend runs on PE as "diagonal matmuls": for each corner,
#     matmul(out=tp[c, pos], lhsT=g_corner[pos, c], rhs=diag(w_corner)) accumulates
#     the weighted corner into PSUM. The diag tiles (identity * per-position folded
#     corner weight) are built by 4x-mode tensor_scalar on DVE (some on ACT), depend
#     only on host-precomputed weights (not the gather), and are allocated in
#     groups of DIAG_GRP per pool tile to amortize semaphore waits.
#   - tp (f32 PSUM) -> sampT (bf16 SBUF) on ACT, then one PSUM accumulator per
#     chunk takes all 18 taps of both branches: the 1x1 fuse conv and the concat
#     are folded into the per-tap weights on the host (W''_k = W_k @ Wf_br^T),
#     so only a bias-add (ACT activation) and the output DMA remain.
import sys

sys.path.insert(0, "/opt/trn_rl_repo")

import numpy as np
import ml_dtypes

import concourse.bass as bass
import concourse.mybir as mybir
from concourse.tile import TileContext
from concourse.masks import make_identity
from concourse import bacc
from concourse.bass_utils import run_bass_kernel_spmd

bf16 = ml_dtypes.bfloat16

# problem constants (hardcoded per spec)
B = 8
C = 128
H = W = 64
HW = H * W                 # 4096
COUT = 84
K = 3
PAD = 1
KK = K * K                 # 9
NBR = 2                    # two deformable branches
# position chunks; small enough that two PSUM accumulators fit (overlapped
# drain) and the final pipeline drain is short, big enough that gather
# descriptor-prep on Pool stays ahead of the DMA transfers
import os as _osmod
_chunks_env = _osmod.environ.get("KERN_CHUNKS", "2048,2048")
CHUNKS = [int(t) for t in _chunks_env.split(",")]
NCH = len(CHUNKS)
CH_OFF = [sum(CHUNKS[:i]) for i in range(NCH)]
NLISTS = NBR * KK * NCH    # gather lists (one per chunk x branch x tap)
IDXCOLS = HW // 16         # idx cols per (br, tap) across all chunks
WSCOLS = (HW // 128) * 4   # ws cols per (br, tap) across all chunks
NSLOT = 2 * 32 * 64        # 4096 parity-packed patch slots

P = 128
f32 = mybir.dt.float32
bft = mybir.dt.bfloat16
i16 = mybir.dt.int16

import os as _os
# diag builds go to ACT when (i * DIAG_ACT_NUM) % DIAG_ACT_DEN rolls under
DIAG_ACT_NUM = int(_os.environ.get("KERN_DIAG_ACT_NUM", "1"))
DIAG_ACT_DEN = int(_os.environ.get("KERN_DIAG_ACT_DEN", "14"))
# tp->sampT copy engine: 1=DVE, 2=ACT (per-copy round robin list)
COPY_ENGS = _os.environ.get("KERN_COPY_ENGS", "2")
GP_BUFS = int(_os.environ.get("KERN_GP_BUFS", "4"))
TPP_BUFS = int(_os.environ.get("KERN_TPP_BUFS", "2"))
SAMP_BUFS = int(_os.environ.get("KERN_SAMP_BUFS", "3"))
DIAG_BUFS = int(_os.environ.get("KERN_DIAG_BUFS", "3"))
DIAG_GRP = int(_os.environ.get("KERN_DIAG_GRP", "16"))  # diags per pool tile
# PSUM budget: out_ps (f32, CHUNKS[0] wide) banks * bufs + tp 2*TPP_BUFS banks
# must fit 8 banks; double-buffer the accumulator only for chunks <= 1024
BIGP_BUFS = int(_os.environ.get("KERN_BIGP_BUFS", "0"))
if BIGP_BUFS == 0:
    BIGP_BUFS = 2 if CHUNKS[0] <= 1024 else 1

_CACHE = {}


def _host_precompute(x, dm0, dm1, w0, w1, wf, bfv):
    """Numpy precompute: patch-slot gather indices + 2D-folded bilinear corner
    weights, parity-packed x, weight repacks."""
    ky = np.repeat(np.arange(K) - PAD, K).astype(np.float32)
    kx = np.tile(np.arange(K) - PAD, K).astype(np.float32)
    base_y = np.arange(H, dtype=np.float32).reshape(1, 1, H, 1)
    base_x = np.arange(W, dtype=np.float32).reshape(1, 1, 1, W)

    idx_all = np.zeros((B, NBR, KK, HW), np.int16)        # patch slot per (tap,pos)
    w_all = np.zeros((B, NBR, KK, 4, HW), np.float32)     # r0c0,r1c0,r0c1,r1c1

    for br, dm in ((0, dm0), (1, dm1)):
        off = dm.reshape(B, KK, 2, H, W)
        py = off[:, :, 0] + base_y + ky.reshape(1, KK, 1, 1)
        px = off[:, :, 1] + base_x + kx.reshape(1, KK, 1, 1)
        y0 = np.floor(py); x0 = np.floor(px)
        wy1 = py - y0; wx1 = px - x0
        wy0 = 1.0 - wy1; wx0 = 1.0 - wx1
        y0i = y0.astype(np.int64); x0i = x0.astype(np.int64)
        yb = np.clip(y0i, 0, H - 2)
        xb = np.clip(x0i, 0, W - 2)
        slot = (yb & 1) * (32 * 64) + (yb >> 1) * 64 + xb
        idx_all[:, br] = slot.reshape(B, KK, HW).astype(np.int16)
        w4 = np.zeros((2, 2) + py.shape, np.float32)      # [rp, cp, B, KK, H, W]
        for r, wy in ((0, wy0), (1, wy1)):
            yi = y0i + r
            rv = ((yi >= 0) & (yi < H)).astype(np.float32)
            rp = np.clip(yi, 0, H - 1) - yb               # 0 or 1
            for c, wx in ((0, wx0), (1, wx1)):
                xi = x0i + c
                cv = ((xi >= 0) & (xi < W)).astype(np.float32)
                cp = np.clip(xi, 0, W - 1) - xb
                contrib = wy * wx * rv * cv
                for rr in (0, 1):
                    for cc in (0, 1):
                        w4[rr, cc] += np.where((rp == rr) & (cp == cc), contrib, 0.0)
        # corner order matches patch byte layout [v00, v10, v01, v11]
        w_all[:, br, :, 0] = w4[0, 0].reshape(B, KK, HW)
        w_all[:, br, :, 1] = w4[1, 0].reshape(B, KK, HW)
        w_all[:, br, :, 2] = w4[0, 1].reshape(B, KK, HW)
        w_all[:, br, :, 3] = w4[1, 1].reshape(B, KK, HW)

    # xPP [B, NSLOT+2, 2C] bf16: slot (par, yy, xx) = rows (2yy+par, 2yy+par+1)
    xhwc = np.transpose(x, (0, 2, 3, 1))                  # [B, H, W, C]
    xPP = np.zeros((B, 2, 32, 64, 2, C), np.float32)
    for par in (0, 1):
        for rp in (0, 1):
            start = par + rp
            rows = xhwc[:, start::2, :, :]                # [B, n, W, C]
            n = min(rows.shape[1], 32)
            xPP[:, par, :n, :, rp, :] = rows[:, :n]
    xPP = xPP.reshape(B, NSLOT, 2 * C)
    xPPp = np.concatenate([xPP, np.zeros((B, 2, 2 * C), np.float32)], axis=1)
    xPPp = xPPp.astype(bf16)

    # IDX [B, 128, totalcols] int16: lists in consumption order (chunk, br, k),
    # each wrapped (j%16, j//16), replicated across the 8 gpsimd cores'
    # 16-partition groups, then concatenated along cols so one DMA loads it.
    # WS [B, 128, totalwcols] f32 likewise (per block: 4 corner weights).
    idx_cols = []
    ws_cols = []
    for ci in range(NCH):
        off, sz = CH_OFF[ci], CHUNKS[ci]
        for br in range(NBR):
            for k in range(KK):
                seq = idx_all[:, br, k, off:off + sz]     # [B, sz]
                wr = np.transpose(seq.reshape(B, sz // 16, 16), (0, 2, 1))
                wr = np.broadcast_to(wr[:, None, :, :], (B, 8, 16, sz // 16))
                idx_cols.append(wr.reshape(B, P, sz // 16))
                wsl = w_all[:, br, k, :, off:off + sz]    # [B, 4, sz]
                wsl = wsl.reshape(B, 4, sz // 128, P)
                wsl = np.transpose(wsl, (0, 3, 2, 1))     # [B, p, blk, c4]
                ws_cols.append(wsl.reshape(B, P, (sz // 128) * 4))
    IDX = np.ascontiguousarray(np.concatenate(idx_cols, axis=2))
    WS = np.ascontiguousarray(np.concatenate(ws_cols, axis=2), np.float32)

    # W0T [128, NTAPS*COUT] bf16: lhsT per (branch, tap) with the 1x1 fuse conv
    # folded in (W''_k = W_k @ Wf_br^T), device layout
    WFT = [wf[:, :COUT, 0, 0].T, wf[:, COUT:, 0, 0].T]    # [84in, 84out] per br
    W0T = np.zeros((NBR * KK, C, COUT), np.float32)
    for br, w in ((0, w0), (1, w1)):
        for k in range(KK):
            W0T[br * KK + k] = w[:, :, k // K, k % K].T @ WFT[br]
    W0T = np.ascontiguousarray(np.transpose(W0T, (1, 0, 2))).astype(bf16)

    BF = bfv.reshape(COUT, 1).astype(np.float32)
    return xPPp, IDX, WS, W0T, BF


def _build_nc():
    nc = bacc.Bacc()
    xpp_d = nc.declare_dram_parameter("xpp", [NSLOT + 2, 2 * C], bft, isOutput=False)
    idx_d = nc.declare_dram_parameter("idx", [P, NBR * KK * IDXCOLS], i16, isOutput=False)
    ws_d = nc.declare_dram_parameter("ws", [P, NBR * KK * WSCOLS], f32, isOutput=False)
    w0_d = nc.declare_dram_parameter("w0t", [C, NBR * KK * COUT], bft, isOutput=False)
    bf_d = nc.declare_dram_parameter("bfv", [COUT, 1], f32, isOutput=False)
    out_d = nc.declare_dram_parameter("out", [COUT, HW], bft, isOutput=True)

    # patch gather source: elem i = slot i (256 elems), read 512 elems (2 slots)
    src_ap = bass.AP(tensor=xpp_d, offset=0, ap=[[2 * C, NSLOT], [1, 4 * C]])

    copy_engs = [int(t) for t in COPY_ENGS.split(",")]

    with TileContext(nc) as tc:
        with tc.tile_pool(name="const", bufs=1) as const, \
             tc.tile_pool(name="gp", bufs=GP_BUFS) as gp, \
             tc.tile_pool(name="dgp", bufs=DIAG_BUFS) as dgp, \
             tc.tile_pool(name="sampp", bufs=SAMP_BUFS) as sampp, \
             tc.tile_pool(name="op", bufs=2) as op, \
             tc.tile_pool(name="tpp", bufs=TPP_BUFS, space="PSUM") as tpp, \
             tc.tile_pool(name="bigp", bufs=BIGP_BUFS, space="PSUM") as bigp:
            ident = const.tile([P, P], bft)
            make_identity(nc, ident[:])
            TIC = NBR * KK * IDXCOLS
            TWC = NBR * KK * WSCOLS
            # split the idx/ws loads so the first gathers launch ASAP
            NIH = 2 * (CHUNKS[0] // 16)
            NWH = 2 * ((CHUNKS[0] // 128) * 4)
            idx_t = const.tile([P, TIC], i16)
            nc.sync.dma_start(out=idx_t[:, 0:NIH], in_=idx_d[:, 0:NIH])
            ws_t = const.tile([P, TWC], f32)
            nc.sync.dma_start(out=ws_t[:, 0:NWH], in_=ws_d[:, 0:NWH])
            nc.sync.dma_start(out=idx_t[:, NIH:], in_=idx_d[:, NIH:TIC])
            nc.sync.dma_start(out=ws_t[:, NWH:], in_=ws_d[:, NWH:TWC])
            w0_t = const.tile([C, NBR * KK, COUT], bft)
            nc.sync.dma_start(out=w0_t[:], in_=w0_d[:])
            bf_t = const.tile([COUT, 1], f32)
            nc.sync.dma_start(out=bf_t[:], in_=bf_d[:])

            # prebuild the final list's diag tiles during the idle startup
            # window (they depend only on ws), so the drain isn't gated on
            # the loaded diag-build pipeline
            nlast = (CHUNKS[-1] // 128) * 4
            dlast = const.tile([P, nlast, P], bft)
            wo_last = TWC - nlast
            for di in range(nlast):
                nc.vector.tensor_scalar(
                    out=dlast[:, di, :], in0=ident[:],
                    scalar1=ws_t[:, wo_last + di:wo_last + di + 1],
                    scalar2=None, op0=mybir.AluOpType.mult,
                )

            ndiag = 0
            ncopy = 0
            io, wo = 0, 0
            dgrp = None
            for ci in range(NCH):
                off, sz = CH_OFF[ci], CHUNKS[ci]
                nb = sz // 128
                out_ps = bigp.tile([COUT, CHUNKS[0]], f32, tag="big")
                for br in range(NBR):
                    for k in range(KK):
                        t = br * KK + k
                        last = (ci == NCH - 1 and br == NBR - 1
                                and k == KK - 1 and sz >= 1024)
                        if last:
                            # split the final gather into separate tiles so
                            # the first half processes while the second half
                            # transfers (deps are tile-granular)
                            hsz = sz // 2
                            ghalves = []
                            for gh in range(2):
                                gt = gp.tile([P, CHUNKS[0] // 256, 4 * C], bft,
                                             tag="glast")
                                nc.gpsimd.dma_gather(
                                    out_ap=gt[:, 0:hsz // 128, :],
                                    in_ap=src_ap,
                                    idxs_ap=idx_t[:, io + gh * (hsz // 16):
                                                  io + (gh + 1) * (hsz // 16)],
                                    num_idxs=hsz, num_idxs_reg=hsz,
                                    elem_size=4 * C, elem_step=2 * C,
                                    transpose=False, single_packet=False,
                                )
                                ghalves.append(gt)
                            g = None
                        else:
                            g = gp.tile([P, CHUNKS[0] // 128, 4 * C], bft,
                                        tag="g")
                            nc.gpsimd.dma_gather(
                                out_ap=g[:, 0:nb, :], in_ap=src_ap,
                                idxs_ap=idx_t[:, io:io + sz // 16],
                                num_idxs=sz, num_idxs_reg=sz,
                                elem_size=4 * C, elem_step=2 * C, transpose=False,
                                single_packet=False,
                            )
                        sampT = sampp.tile([C, CHUNKS[0]], bft, tag="sampT")
                        stripb = 4 if last else 8
                        for qh in range((nb + stripb - 1) // stripb):
                            qnb = min(stripb, nb - qh * stripb)
                            tp = tpp.tile([C, 1024], f32, tag="tp")
                            for jb in range(qnb):
                                b = qh * stripb + jb
                                for c4 in range(4):
                                    if last:
                                        diag = dlast[:, b * 4 + c4, :]
                                    else:
                                        gi = ndiag % DIAG_GRP
                                        if gi == 0:
                                            dgrp = dgp.tile([P, DIAG_GRP, P],
                                                            bft, tag="diag")
                                        diag = dgrp[:, gi, :]
                                        sc = ws_t[:, wo + b * 4 + c4:
                                                  wo + b * 4 + c4 + 1]
                                        on_act = (ndiag * DIAG_ACT_NUM) \
                                            % DIAG_ACT_DEN < DIAG_ACT_NUM
                                        ndiag += 1
                                        if on_act:
                                            nc.scalar.activation(
                                                out=diag, in_=ident[:],
                                                func=mybir.ActivationFunctionType.Identity,
                                                scale=sc,
                                            )
                                        else:
                                            nc.vector.tensor_scalar(
                                                out=diag, in0=ident[:],
                                                scalar1=sc, scalar2=None,
                                                op0=mybir.AluOpType.mult,
                                            )
                                    if last:
                                        hb = sz // 256
                                        gsrc = ghalves[b // hb][:, b % hb, :]
                                    else:
                                        gsrc = g[:, b, :]
                                    nc.tensor.matmul(
                                        out=tp[:, jb * P:(jb + 1) * P],
                                        lhsT=gsrc[:, c4 * C:(c4 + 1) * C],
                                        rhs=diag,
                                        start=(c4 == 0), stop=(c4 == 3),
                                    )
                            ce = copy_engs[ncopy % len(copy_engs)]
                            ncopy += 1
                            qoff = qh * stripb * P
                            dst = sampT[:, qoff:qoff + qnb * P]
                            if ce == 1:
                                nc.vector.tensor_copy(out=dst,
                                                      in_=tp[:, 0:qnb * P])
                            else:
                                nc.scalar.copy(out=dst, in_=tp[:, 0:qnb * P])
                            if last:
                                # deform per 512-strip so the drain pipelines
                                nc.tensor.matmul(
                                    out=out_ps[:, qoff:qoff + qnb * P],
                                    lhsT=w0_t[:, t, :],
                                    rhs=sampT[:, qoff:qoff + qnb * P],
                                    start=False, stop=True,
                                )
                                if qh == sz // 1024 - 1:
                                    # first half fully accumulated: drain it
                                    # while the remaining strips process
                                    out_sb = op.tile([COUT, CHUNKS[0]], bft,
                                                     tag="outsb")
                                    nc.scalar.activation(
                                        out=out_sb[:, 0:sz // 2],
                                        in_=out_ps[:, 0:sz // 2],
                                        func=mybir.ActivationFunctionType.Identity,
                                        bias=bf_t[:], scale=1.0,
                                    )
                                    nc.sync.dma_start(
                                        out=out_d[:, off:off + sz // 2],
                                        in_=out_sb[:, 0:sz // 2])
                        if not last:
                            for cs in range(0, sz, 512):
                                ce_ = min(cs + 512, sz)
                                nc.tensor.matmul(
                                    out=out_ps[:, cs:ce_],
                                    lhsT=w0_t[:, t, :],
                                    rhs=sampT[:, cs:ce_],
                                    start=(br == 0 and k == 0),
                                    stop=(br == NBR - 1 and k == KK - 1),
                                )
                        io += sz // 16
                        wo += (sz // 128) * 4
                out_sb = op.tile([COUT, CHUNKS[0]], bft, tag="outsb")
                if ci == NCH - 1 and sz >= 1024:
                    # second half only: the first half drained mid-strip-loop
                    nc.scalar.activation(
                        out=out_sb[:, sz // 2:sz], in_=out_ps[:, sz // 2:sz],
                        func=mybir.ActivationFunctionType.Identity,
                        bias=bf_t[:], scale=1.0,
                    )
                    nc.sync.dma_start(out=out_d[:, off + sz // 2:off + sz],
                                      in_=out_sb[:, sz // 2:sz])
                else:
                    nc.scalar.activation(
                        out=out_sb[:, 0:sz], in_=out_ps[:, 0:sz],
                        func=mybir.ActivationFunctionType.Identity, bias=bf_t[:],
                        scale=1.0,
                    )
                    nc.sync.dma_start(out=out_d[:, off:off + sz],
                                      in_=out_sb[:, 0:sz])
    nc.finalize()
    return nc


def kernel(x, dm0, dm1, w0, w1, wf, bf):
    x = np.asarray(x, np.float32)
    dm0 = np.asarray(dm0, np.float32)
    dm1 = np.asarray(dm1, np.float32)
    w0 = np.asarray(w0, np.float32)
    w1 = np.asarray(w1, np.float32)
    wf = np.asarray(wf, np.float32)
    bfv = np.asarray(bf, np.float32)

    xPPp, IDX, WS, W0T, BF = _host_precompute(x, dm0, dm1, w0, w1, wf, bfv)

    if "nc" not in _CACHE:
        _CACHE["nc"] = _build_nc()
    nc = _CACHE["nc"]

    in_maps = [
        {
            "xpp": np.ascontiguousarray(xPPp[i]),
            "idx": np.ascontiguousarray(IDX[i].reshape(P, -1)),
            "ws": np.ascontiguousarray(WS[i].reshape(P, -1)),
            "w0t": W0T.reshape(C, -1),
            "bfv": BF,
        }
        for i in range(B)
    ]
    res = run_bass_kernel_spmd(nc, in_maps, core_ids=list(range(B)),
                               **_CACHE.get("run_kwargs", {}))
    _CACHE["last_results"] = res
    out = np.stack([np.asarray(res.results[i]["out"], np.float32)
                    for i in range(B)])
    return out.reshape(B, COUT, H, W)


# revision 53
# speedup vs baseline: 1.0076x; 1.0003x over previous
# Trainium2 Bass kernel for nn_DeformableInception (deformable conv x2 -> concat -> 1x1 conv).
#
# Sharding: data-parallel over batch B=8, one sample per NeuronCore (8 cores).
# Weights replicated. No collectives.
#
# Per-core device pipeline (per sample):
#   - x is stored in DRAM as parity-packed row pairs: slot (par, yy, xx) holds
#     image rows (2*yy+par, 2*yy+par+1) x 128ch bf16 (512B). A bilinear 2x2 patch
#     at (yb, xb) is two adjacent slots = ONE contiguous 1KB gather descriptor
#     (>=512B, so no DMA read-modify-write penalty).
#   - per (chunk, branch, tap): SWDGE dma_gather fetches one 1KB patch per output
#     position (positions land on partitions): g[pos, blk, 512] = [v00|v10|v01|v11].
#   - the bilinear blend runs on PE as "diagonal matmuls": for each corner,
#     matmul(out=tp[c, pos], lhsT=g_corner[pos, c], rhs=diag(w_corner)) accumulates
#     the weighted corner into PSUM. The diag tiles (identity * per-position folded
#     corner weight) are built by 4x-mode tensor_scalar on DVE (some on ACT), depend
#     only on host-precomputed weights (not the gather), and are allocated in
#     groups of DIAG_GRP per pool tile to amortize semaphore waits.
#   - tp (f32 PSUM) -> sampT (bf16 SBUF) on ACT, then one PSUM accumulator per
#     chunk takes all 18 taps of both branches: the 1x1 fuse conv and the concat
#     are folded into the per-tap weights on the host (W''_k = W_k @ Wf_br^T),
#     so only a bias-add (ACT activation) and the output DMA remain.
import sys

sys.path.insert(0, "/opt/trn_rl_repo")

import numpy as np
import ml_dtypes

import concourse.bass as bass
import concourse.mybir as mybir
from concourse.tile import TileContext
from concourse.masks import make_identity
from concourse import bacc
from concourse.bass_utils import run_bass_kernel_spmd

bf16 = ml_dtypes.bfloat16

# problem constants (hardcoded per spec)
B = 8
C = 128
H = W = 64
HW = H * W                 # 4096
COUT = 84
K = 3
PAD = 1
KK = K * K                 # 9
NBR = 2                    # two deformable branches
# position chunks; small enough that two PSUM accumulators fit (overlapped
# drain) and the final pipeline drain is short, big enough that gather
# descriptor-prep on Pool stays ahead of the DMA transfers
import os as _osmod
_chunks_env = _osmod.environ.get("KERN_CHUNKS", "2048,2048")
CHUNKS = [int(t) for t in _chunks_env.split(",")]
NCH = len(CHUNKS)
CH_OFF = [sum(CHUNKS[:i]) for i in range(NCH)]
NLISTS = NBR * KK * NCH    # gather lists (one per chunk x branch x tap)
IDXCOLS = HW // 16         # idx cols per (br, tap) across all chunks
WSCOLS = (HW // 128) * 4   # ws cols per (br, tap) across all chunks
NSLOT = 2 * 32 * 64        # 4096 parity-packed patch slots

P = 128
f32 = mybir.dt.float32
bft = mybir.dt.bfloat16
i16 = mybir.dt.int16

import os as _os
# diag builds go to ACT when (i * DIAG_ACT_NUM) % DIAG_ACT_DEN rolls under
DIAG_ACT_NUM = int(_os.environ.get("KERN_DIAG_ACT_NUM", "1"))
DIAG_ACT_DEN = int(_os.environ.get("KERN_DIAG_ACT_DEN", "14"))
# tp->sampT copy engine: 1=DVE, 2=ACT (per-copy round robin list)
COPY_ENGS = _os.environ.get("KERN_COPY_ENGS", "2")
GP_BUFS = int(_os.environ.get("KERN_GP_BUFS", "4"))
TPP_BUFS = int(_os.environ.get("KERN_TPP_BUFS", "2"))
SAMP_BUFS = int(_os.environ.get("KERN_SAMP_BUFS", "3"))
DIAG_BUFS = int(_os.environ.get("KERN_DIAG_BUFS", "3"))
DIAG_GRP = int(_os.environ.get("KERN_DIAG_GRP", "16"))  # diags per pool tile
# PSUM budget: out_ps (f32, CHUNKS[0] wide) banks * bufs + tp 2*TPP_BUFS banks
# must fit 8 banks; double-buffer the accumulator only for chunks <= 1024
BIGP_BUFS = int(_os.environ.get("KERN_BIGP_BUFS", "0"))
if BIGP_BUFS == 0:
    BIGP_BUFS = 2 if CHUNKS[0] <= 1024 else 1

_CACHE = {}


def _host_precompute(x, dm0, dm1, w0, w1, wf, bfv):
    """Numpy precompute: patch-slot gather indices + 2D-folded bilinear corner
    weights, parity-packed x, weight repacks."""
    ky = np.repeat(np.arange(K) - PAD, K).astype(np.float32)
    kx = np.tile(np.arange(K) - PAD, K).astype(np.float32)
    base_y = np.arange(H, dtype=np.float32).reshape(1, 1, H, 1)
    base_x = np.arange(W, dtype=np.float32).reshape(1, 1, 1, W)

    idx_all = np.zeros((B, NBR, KK, HW), np.int16)        # patch slot per (tap,pos)
    w_all = np.zeros((B, NBR, KK, 4, HW), np.float32)     # r0c0,r1c0,r0c1,r1c1

    for br, dm in ((0, dm0), (1, dm1)):
        off = dm.reshape(B, KK, 2, H, W)
        py = off[:, :, 0] + base_y + ky.reshape(1, KK, 1, 1)
        px = off[:, :, 1] + base_x + kx.reshape(1, KK, 1, 1)
        y0 = np.floor(py); x0 = np.floor(px)
        wy1 = py - y0; wx1 = px - x0
        wy0 = 1.0 - wy1; wx0 = 1.0 - wx1
        y0i = y0.astype(np.int64); x0i = x0.astype(np.int64)
        yb = np.clip(y0i, 0, H - 2)
        xb = np.clip(x0i, 0, W - 2)
        slot = (yb & 1) * (32 * 64) + (yb >> 1) * 64 + xb
        idx_all[:, br] = slot.reshape(B, KK, HW).astype(np.int16)
        w4 = np.zeros((2, 2) + py.shape, np.float32)      # [rp, cp, B, KK, H, W]
        for r, wy in ((0, wy0), (1, wy1)):
            yi = y0i + r
            rv = ((yi >= 0) & (yi < H)).astype(np.float32)
            rp = np.clip(yi, 0, H - 1) - yb               # 0 or 1
            for c, wx in ((0, wx0), (1, wx1)):
                xi = x0i + c
                cv = ((xi >= 0) & (xi < W)).astype(np.float32)
                cp = np.clip(xi, 0, W - 1) - xb
                contrib = wy * wx * rv * cv
                for rr in (0, 1):
                    for cc in (0, 1):
                        w4[rr, cc] += np.where((rp == rr) & (cp == cc), contrib, 0.0)
        # corner order matches patch byte layout [v00, v10, v01, v11]
        w_all[:, br, :, 0] = w4[0, 0].reshape(B, KK, HW)
        w_all[:, br, :, 1] = w4[1, 0].reshape(B, KK, HW)
        w_all[:, br, :, 2] = w4[0, 1].reshape(B, KK, HW)
        w_all[:, br, :, 3] = w4[1, 1].reshape(B, KK, HW)

    # xPP [B, NSLOT+2, 2C] bf16: slot (par, yy, xx) = rows (2yy+par, 2yy+par+1)
    xhwc = np.transpose(x, (0, 2, 3, 1))                  # [B, H, W, C]
    xPP = np.zeros((B, 2, 32, 64, 2, C), np.float32)
    for par in (0, 1):
        for rp in (0, 1):
            start = par + rp
            rows = xhwc[:, start::2, :, :]                # [B, n, W, C]
            n = min(rows.shape[1], 32)
            xPP[:, par, :n, :, rp, :] = rows[:, :n]
    xPP = xPP.reshape(B, NSLOT, 2 * C)
    xPPp = np.concatenate([xPP, np.zeros((B, 2, 2 * C), np.float32)], axis=1)
    xPPp = xPPp.astype(bf16)

    # IDX [B, 128, totalcols] int16: lists in consumption order (chunk, br, k),
    # each wrapped (j%16, j//16), replicated across the 8 gpsimd cores'
    # 16-partition groups, then concatenated along cols so one DMA loads it.
    # WS [B, 128, totalwcols] f32 likewise (per block: 4 corner weights).
    idx_cols = []
    ws_cols = []
    for ci in range(NCH):
        off, sz = CH_OFF[ci], CHUNKS[ci]
        for br in range(NBR):
            for k in range(KK):
                seq = idx_all[:, br, k, off:off + sz]     # [B, sz]
                wr = np.transpose(seq.reshape(B, sz // 16, 16), (0, 2, 1))
                wr = np.broadcast_to(wr[:, None, :, :], (B, 8, 16, sz // 16))
                idx_cols.append(wr.reshape(B, P, sz // 16))
                wsl = w_all[:, br, k, :, off:off + sz]    # [B, 4, sz]
                wsl = wsl.reshape(B, 4, sz // 128, P)
                wsl = np.transpose(wsl, (0, 3, 2, 1))     # [B, p, blk, c4]
                ws_cols.append(wsl.reshape(B, P, (sz // 128) * 4))
    IDX = np.ascontiguousarray(np.concatenate(idx_cols, axis=2))
    WS = np.ascontiguousarray(np.concatenate(ws_cols, axis=2), np.float32)

    # W0T [128, NTAPS*COUT] bf16: lhsT per (branch, tap) with the 1x1 fuse conv
    # folded in (W''_k = W_k @ Wf_br^T), device layout
    WFT = [wf[:, :COUT, 0, 0].T, wf[:, COUT:, 0, 0].T]    # [84in, 84out] per br
    W0T = np.zeros((NBR * KK, C, COUT), np.float32)
    for br, w in ((0, w0), (1, w1)):
        for k in range(KK):
            W0T[br * KK + k] = w[:, :, k // K, k % K].T @ WFT[br]
    W0T = np.ascontiguousarray(np.transpose(W0T, (1, 0, 2))).astype(bf16)

    BF = bfv.reshape(COUT, 1).astype(np.float32)
    return xPPp, IDX, WS, W0T, BF


def _build_nc():
    nc = bacc.Bacc()
    xpp_d = nc.declare_dram_parameter("xpp", [NSLOT + 2, 2 * C], bft, isOutput=False)
    idx_d = nc.declare_dram_parameter("idx", [P, NBR * KK * IDXCOLS], i16, isOutput=False)
    ws_d = nc.declare_dram_parameter("ws", [P, NBR * KK * WSCOLS], f32, isOutput=False)
    w0_d = nc.declare_dram_parameter("w0t", [C, NBR * KK * COUT], bft, isOutput=False)
    bf_d = nc.declare_dram_parameter("bfv", [COUT, 1], f32, isOutput=False)
    out_d = nc.declare_dram_parameter("out", [COUT, HW], bft, isOutput=True)

    # patch gather source: elem i = slot i (256 elems), read 512 elems (2 slots)
    src_ap = bass.AP(tensor=xpp_d, offset=0, ap=[[2 * C, NSLOT], [1, 4 * C]])

    copy_engs = [int(t) for t in COPY_ENGS.split(",")]

    with TileContext(nc) as tc:
        with tc.tile_pool(name="const", bufs=1) as const, \
             tc.tile_pool(name="gp", bufs=GP_BUFS) as gp, \
             tc.tile_pool(name="dgp", bufs=DIAG_BUFS) as dgp, \
             tc.tile_pool(name="sampp", bufs=SAMP_BUFS) as sampp, \
             tc.tile_pool(name="op", bufs=2) as op, \
             tc.tile_pool(name="tpp", bufs=TPP_BUFS, space="PSUM") as tpp, \
             tc.tile_pool(name="bigp", bufs=BIGP_BUFS, space="PSUM") as bigp:
            ident = const.tile([P, P], bft)
            make_identity(nc, ident[:])
            TIC = NBR * KK * IDXCOLS
            TWC = NBR * KK * WSCOLS
            # split the idx/ws loads so the first gathers launch ASAP
            NIH = 2 * (CHUNKS[0] // 16)
            NWH = 2 * ((CHUNKS[0] // 128) * 4)
            idx_t = const.tile([P, TIC], i16)
            nc.sync.dma_start(out=idx_t[:, 0:NIH], in_=idx_d[:, 0:NIH])
            ws_t = const.tile([P, TWC], f32)
            nc.sync.dma_start(out=ws_t[:, 0:NWH], in_=ws_d[:, 0:NWH])
            nc.sync.dma_start(out=idx_t[:, NIH:], in_=idx_d[:, NIH:TIC])
            nc.sync.dma_start(out=ws_t[:, NWH:], in_=ws_d[:, NWH:TWC])
            w0_t = const.tile([C, NBR * KK, COUT], bft)
            nc.sync.dma_start(out=w0_t[:], in_=w0_d[:])
            bf_t = const.tile([COUT, 1], f32)
            nc.sync.dma_start(out=bf_t[:], in_=bf_d[:])

            # prebuild the final list's diag tiles during the idle startup
            # window (they depend only on ws), so the drain isn't gated on
            # the loaded diag-build pipeline
            nlast = (CHUNKS[-1] // 128) * 4
            dlast = const.tile([P, nlast, P], bft)
            wo_last = TWC - nlast
            for di in range(nlast):
                nc.vector.tensor_scalar(
                    out=dlast[:, di, :], in0=ident[:],
                    scalar1=ws_t[:, wo_last + di:wo_last + di + 1],
                    scalar2=None, op0=mybir.AluOpType.mult,
                )

            ndiag = 0
            ncopy = 0
            io, wo = 0, 0
            dgrp = None
            for ci in range(NCH):
                off, sz = CH_OFF[ci], CHUNKS[ci]
                nb = sz // 128
                out_ps = bigp.tile([COUT, CHUNKS[0]], f32, tag="big")
                for br in range(NBR):
                    for k in range(KK):
                        t = br * KK + k
                        last = (ci == NCH - 1 and br == NBR - 1
                                and k == KK - 1 and sz >= 1024)
                        if last:
                            # split the final gather into separate tiles so
                            # the first half processes while the second half
                            # transfers (deps are tile-granular)
                            hsz = sz // 2
                            ghalves = []
                            for gh in range(2):
                                gt = gp.tile([P, CHUNKS[0] // 256, 4 * C], bft,
                                             tag="glast")
                                nc.gpsimd.dma_gather(
                                    out_ap=gt[:, 0:hsz // 128, :],
                                    in_ap=src_ap,
                                    idxs_ap=idx_t[:, io + gh * (hsz // 16):
                                                  io + (gh + 1) * (hsz // 16)],
                                    num_idxs=hsz, num_idxs_reg=hsz,
                                    elem_size=4 * C, elem_step=2 * C,
                                    transpose=False, single_packet=False,
                                )
                                ghalves.append(gt)
                            g = None
                        else:
                            g = gp.tile([P, CHUNKS[0] // 128, 4 * C], bft,
                                        tag="g")
                            nc.gpsimd.dma_gather(
                                out_ap=g[:, 0:nb, :], in_ap=src_ap,
                                idxs_ap=idx_t[:, io:io + sz // 16],
                                num_idxs=sz, num_idxs_reg=sz,
                                elem_size=4 * C, elem_step=2 * C, transpose=False,
                                single_packet=False,
                            )
                        sampT = sampp.tile([C, CHUNKS[0]], bft, tag="sampT")
                        stripb = 4 if last else 8
                        for qh in range((nb + stripb - 1) // stripb):
                            qnb = min(stripb, nb - qh * stripb)
                            tp = tpp.tile([C, 1024], f32, tag="tp")
                            for jb in range(qnb):
                                b = qh * stripb + jb
                                for c4 in range(4):
                                    if last:
                                        diag = dlast[:, b * 4 + c4, :]
                                    else:
                                        gi = ndiag % DIAG_GRP
                                        if gi == 0:
                                            dgrp = dgp.tile([P, DIAG_GRP, P],
                                                            bft, tag="diag")
                                        diag = dgrp[:, gi, :]
                                        sc = ws_t[:, wo + b * 4 + c4:
                                                  wo + b * 4 + c4 + 1]
                                        on_act = (ndiag * DIAG_ACT_NUM) \
                                            % DIAG_ACT_DEN < DIAG_ACT_NUM
                                        ndiag += 1
                                        if on_act:
                                            nc.scalar.activation(
                                                out=diag, in_=ident[:],
                                                func=mybir.ActivationFunctionType.Identity,
                                                scale=sc,
                                            )
                                        else:
                                            nc.vector.tensor_scalar(
                                                out=diag, in0=ident[:],
                                                scalar1=sc, scalar2=None,
                                                op0=mybir.AluOpType.mult,
                                            )
                                    if last:
                                        hb = sz // 256
                                        gsrc = ghalves[b // hb][:, b % hb, :]
                                    else:
                                        gsrc = g[:, b, :]
                                    nc.tensor.matmul(
                                        out=tp[:, jb * P:(jb + 1) * P],
                                        lhsT=gsrc[:, c4 * C:(c4 + 1) * C],
                                        rhs=diag,
                                        start=(c4 == 0), stop=(c4 == 3),
                                    )
                            if ci == NCH - 1 and br == NBR - 1 and k >= KK - 1:
                                # drain region: DVE is idle, ACT is the
                                # chokepoint — move the tail evacs to DVE
                                ce = 1
                            else:
                                ce = copy_engs[ncopy % len(copy_engs)]
                            ncopy += 1
                            qoff = qh * stripb * P
                            dst = sampT[:, qoff:qoff + qnb * P]
                            if ce == 1:
                                nc.vector.tensor_copy(out=dst,
                                                      in_=tp[:, 0:qnb * P])
                            else:
                                nc.scalar.copy(out=dst, in_=tp[:, 0:qnb * P])
                            if last:
                                # deform per 512-strip so the drain pipelines
                                nc.tensor.matmul(
                                    out=out_ps[:, qoff:qoff + qnb * P],
                                    lhsT=w0_t[:, t, :],
                                    rhs=sampT[:, qoff:qoff + qnb * P],
                                    start=False, stop=True,
                                )
                                if qh == sz // 1024 - 1:
                                    # first half fully accumulated: drain it
                                    # while the remaining strips process
                                    out_sb = op.tile([COUT, CHUNKS[0]], bft,
                                                     tag="outsb")
                                    nc.scalar.activation(
                                        out=out_sb[:, 0:sz // 2],
                                        in_=out_ps[:, 0:sz // 2],
                                        func=mybir.ActivationFunctionType.Identity,
                                        bias=bf_t[:], scale=1.0,
                                    )
                                    nc.sync.dma_start(
                                        out=out_d[:, off:off + sz // 2],
                                        in_=out_sb[:, 0:sz // 2])
                        if not last:
                            for cs in range(0, sz, 512):
                                ce_ = min(cs + 512, sz)
                                nc.tensor.matmul(
                                    out=out_ps[:, cs:ce_],
                                    lhsT=w0_t[:, t, :],
                                    rhs=sampT[:, cs:ce_],
                                    start=(br == 0 and k == 0),
                                    stop=(br == NBR - 1 and k == KK - 1),
                                )
                        io += sz // 16
                        wo += (sz // 128) * 4
                out_sb = op.tile([COUT, CHUNKS[0]], bft, tag="outsb")
                if ci == NCH - 1 and sz >= 1024:
                    # second half only: the first half drained mid-strip-loop
                    nc.scalar.activation(
                        out=out_sb[:, sz // 2:sz], in_=out_ps[:, sz // 2:sz],
                        func=mybir.ActivationFunctionType.Identity,
                        bias=bf_t[:], scale=1.0,
                    )
                    nc.sync.dma_start(out=out_d[:, off + sz // 2:off + sz],
                                      in_=out_sb[:, sz // 2:sz])
                else:
                    nc.scalar.activation(
                        out=out_sb[:, 0:sz], in_=out_ps[:, 0:sz],
                        func=mybir.ActivationFunctionType.Identity, bias=bf_t[:],
                        scale=1.0,
                    )
                    nc.sync.dma_start(out=out_d[:, off:off + sz],
                                      in_=out_sb[:, 0:sz])
    nc.finalize()
    return nc


def kernel(x, dm0, dm1, w0, w1, wf, bf):
    x = np.asarray(x, np.float32)
    dm0 = np.asarray(dm0, np.float32)
    dm1 = np.asarray(dm1, np.float32)
    w0 = np.asarray(w0, np.float32)
    w1 = np.asarray(w1, np.float32)
    wf = np.asarray(wf, np.float32)
    bfv = np.asarray(bf, np.float32)

    xPPp, IDX, WS, W0T, BF = _host_precompute(x, dm0, dm1, w0, w1, wf, bfv)

    if "nc" not in _CACHE:
        _CACHE["nc"] = _build_nc()
    nc = _CACHE["nc"]

    in_maps = [
        {
            "xpp": np.ascontiguousarray(xPPp[i]),
            "idx": np.ascontiguousarray(IDX[i].reshape(P, -1)),
            "ws": np.ascontiguousarray(WS[i].reshape(P, -1)),
            "w0t": W0T.reshape(C, -1),
            "bfv": BF,
        }
        for i in range(B)
    ]
    res = run_bass_kernel_spmd(nc, in_maps, core_ids=list(range(B)),
                               **_CACHE.get("run_kwargs", {}))
    _CACHE["last_results"] = res
    out = np.stack([np.asarray(res.results[i]["out"], np.float32)
                    for i in range(B)])
    return out.reshape(B, COUT, H, W)


# revision 54
# speedup vs baseline: 1.0143x; 1.0066x over previous
# Trainium2 Bass kernel for nn_DeformableInception (deformable conv x2 -> concat -> 1x1 conv).
#
# Sharding: data-parallel over batch B=8, one sample per NeuronCore (8 cores).
# Weights replicated. No collectives.
#
# Per-core device pipeline (per sample):
#   - x is stored in DRAM as parity-packed row pairs: slot (par, yy, xx) holds
#     image rows (2*yy+par, 2*yy+par+1) x 128ch bf16 (512B). A bilinear 2x2 patch
#     at (yb, xb) is two adjacent slots = ONE contiguous 1KB gather descriptor
#     (>=512B, so no DMA read-modify-write penalty).
#   - per (chunk, branch, tap): SWDGE dma_gather fetches one 1KB patch per output
#     position (positions land on partitions): g[pos, blk, 512] = [v00|v10|v01|v11].
#   - the bilinear blend runs on PE as "diagonal matmuls": for each corner,
#     matmul(out=tp[c, pos], lhsT=g_corner[pos, c], rhs=diag(w_corner)) accumulates
#     the weighted corner into PSUM. The diag tiles (identity * per-position folded
#     corner weight) are built by 4x-mode tensor_scalar on DVE (some on ACT), depend
#     only on host-precomputed weights (not the gather), and are allocated in
#     groups of DIAG_GRP per pool tile to amortize semaphore waits.
#   - tp (f32 PSUM) -> sampT (bf16 SBUF) on ACT, then one PSUM accumulator per
#     chunk takes all 18 taps of both branches: the 1x1 fuse conv and the concat
#     are folded into the per-tap weights on the host (W''_k = W_k @ Wf_br^T),
#     so only a bias-add (ACT activation) and the output DMA remain.
import sys

sys.path.insert(0, "/opt/trn_rl_repo")

import numpy as np
import ml_dtypes

import concourse.bass as bass
import concourse.mybir as mybir
from concourse.tile import TileContext
from concourse.masks import make_identity
from concourse import bacc
from concourse.bass_utils import run_bass_kernel_spmd

bf16 = ml_dtypes.bfloat16

# problem constants (hardcoded per spec)
B = 8
C = 128
H = W = 64
HW = H * W                 # 4096
COUT = 84
K = 3
PAD = 1
KK = K * K                 # 9
NBR = 2                    # two deformable branches
# position chunks; small enough that two PSUM accumulators fit (overlapped
# drain) and the final pipeline drain is short, big enough that gather
# descriptor-prep on Pool stays ahead of the DMA transfers
import os as _osmod
_chunks_env = _osmod.environ.get("KERN_CHUNKS", "2048,2048")
CHUNKS = [int(t) for t in _chunks_env.split(",")]
NCH = len(CHUNKS)
CH_OFF = [sum(CHUNKS[:i]) for i in range(NCH)]
NLISTS = NBR * KK * NCH    # gather lists (one per chunk x branch x tap)
IDXCOLS = HW // 16         # idx cols per (br, tap) across all chunks
WSCOLS = (HW // 128) * 4   # ws cols per (br, tap) across all chunks
NSLOT = 2 * 32 * 64        # 4096 parity-packed patch slots

P = 128
f32 = mybir.dt.float32
bft = mybir.dt.bfloat16
i16 = mybir.dt.int16

import os as _os
# diag builds go to ACT when (i * DIAG_ACT_NUM) % DIAG_ACT_DEN rolls under
DIAG_ACT_NUM = int(_os.environ.get("KERN_DIAG_ACT_NUM", "1"))
DIAG_ACT_DEN = int(_os.environ.get("KERN_DIAG_ACT_DEN", "14"))
# tp->sampT copy engine: 1=DVE, 2=ACT (per-copy round robin list)
COPY_ENGS = _os.environ.get("KERN_COPY_ENGS", "2")
GP_BUFS = int(_os.environ.get("KERN_GP_BUFS", "4"))
TPP_BUFS = int(_os.environ.get("KERN_TPP_BUFS", "2"))
SAMP_BUFS = int(_os.environ.get("KERN_SAMP_BUFS", "3"))
DIAG_BUFS = int(_os.environ.get("KERN_DIAG_BUFS", "3"))
DIAG_GRP = int(_os.environ.get("KERN_DIAG_GRP", "16"))  # diags per pool tile
# PSUM budget: out_ps (f32, CHUNKS[0] wide) banks * bufs + tp 2*TPP_BUFS banks
# must fit 8 banks; double-buffer the accumulator only for chunks <= 1024
PREBUILD_LISTS = int(_os.environ.get("KERN_PREBUILD_LISTS", "2"))
BIGP_BUFS = int(_os.environ.get("KERN_BIGP_BUFS", "0"))
if BIGP_BUFS == 0:
    BIGP_BUFS = 2 if CHUNKS[0] <= 1024 else 1

_CACHE = {}


def _host_precompute(x, dm0, dm1, w0, w1, wf, bfv):
    """Numpy precompute: patch-slot gather indices + 2D-folded bilinear corner
    weights, parity-packed x, weight repacks."""
    ky = np.repeat(np.arange(K) - PAD, K).astype(np.float32)
    kx = np.tile(np.arange(K) - PAD, K).astype(np.float32)
    base_y = np.arange(H, dtype=np.float32).reshape(1, 1, H, 1)
    base_x = np.arange(W, dtype=np.float32).reshape(1, 1, 1, W)

    idx_all = np.zeros((B, NBR, KK, HW), np.int16)        # patch slot per (tap,pos)
    w_all = np.zeros((B, NBR, KK, 4, HW), np.float32)     # r0c0,r1c0,r0c1,r1c1

    for br, dm in ((0, dm0), (1, dm1)):
        off = dm.reshape(B, KK, 2, H, W)
        py = off[:, :, 0] + base_y + ky.reshape(1, KK, 1, 1)
        px = off[:, :, 1] + base_x + kx.reshape(1, KK, 1, 1)
        y0 = np.floor(py); x0 = np.floor(px)
        wy1 = py - y0; wx1 = px - x0
        wy0 = 1.0 - wy1; wx0 = 1.0 - wx1
        y0i = y0.astype(np.int64); x0i = x0.astype(np.int64)
        yb = np.clip(y0i, 0, H - 2)
        xb = np.clip(x0i, 0, W - 2)
        slot = (yb & 1) * (32 * 64) + (yb >> 1) * 64 + xb
        idx_all[:, br] = slot.reshape(B, KK, HW).astype(np.int16)
        w4 = np.zeros((2, 2) + py.shape, np.float32)      # [rp, cp, B, KK, H, W]
        for r, wy in ((0, wy0), (1, wy1)):
            yi = y0i + r
            rv = ((yi >= 0) & (yi < H)).astype(np.float32)
            rp = np.clip(yi, 0, H - 1) - yb               # 0 or 1
            for c, wx in ((0, wx0), (1, wx1)):
                xi = x0i + c
                cv = ((xi >= 0) & (xi < W)).astype(np.float32)
                cp = np.clip(xi, 0, W - 1) - xb
                contrib = wy * wx * rv * cv
                for rr in (0, 1):
                    for cc in (0, 1):
                        w4[rr, cc] += np.where((rp == rr) & (cp == cc), contrib, 0.0)
        # corner order matches patch byte layout [v00, v10, v01, v11]
        w_all[:, br, :, 0] = w4[0, 0].reshape(B, KK, HW)
        w_all[:, br, :, 1] = w4[1, 0].reshape(B, KK, HW)
        w_all[:, br, :, 2] = w4[0, 1].reshape(B, KK, HW)
        w_all[:, br, :, 3] = w4[1, 1].reshape(B, KK, HW)

    # xPP [B, NSLOT+2, 2C] bf16: slot (par, yy, xx) = rows (2yy+par, 2yy+par+1)
    xhwc = np.transpose(x, (0, 2, 3, 1))                  # [B, H, W, C]
    xPP = np.zeros((B, 2, 32, 64, 2, C), np.float32)
    for par in (0, 1):
        for rp in (0, 1):
            start = par + rp
            rows = xhwc[:, start::2, :, :]                # [B, n, W, C]
            n = min(rows.shape[1], 32)
            xPP[:, par, :n, :, rp, :] = rows[:, :n]
    xPP = xPP.reshape(B, NSLOT, 2 * C)
    xPPp = np.concatenate([xPP, np.zeros((B, 2, 2 * C), np.float32)], axis=1)
    xPPp = xPPp.astype(bf16)

    # IDX [B, 128, totalcols] int16: lists in consumption order (chunk, br, k),
    # each wrapped (j%16, j//16), replicated across the 8 gpsimd cores'
    # 16-partition groups, then concatenated along cols so one DMA loads it.
    # WS [B, 128, totalwcols] f32 likewise (per block: 4 corner weights).
    idx_cols = []
    ws_cols = []
    for ci in range(NCH):
        off, sz = CH_OFF[ci], CHUNKS[ci]
        for br in range(NBR):
            for k in range(KK):
                seq = idx_all[:, br, k, off:off + sz]     # [B, sz]
                wr = np.transpose(seq.reshape(B, sz // 16, 16), (0, 2, 1))
                wr = np.broadcast_to(wr[:, None, :, :], (B, 8, 16, sz // 16))
                idx_cols.append(wr.reshape(B, P, sz // 16))
                wsl = w_all[:, br, k, :, off:off + sz]    # [B, 4, sz]
                wsl = wsl.reshape(B, 4, sz // 128, P)
                wsl = np.transpose(wsl, (0, 3, 2, 1))     # [B, p, blk, c4]
                ws_cols.append(wsl.reshape(B, P, (sz // 128) * 4))
    IDX = np.ascontiguousarray(np.concatenate(idx_cols, axis=2))
    WS = np.ascontiguousarray(np.concatenate(ws_cols, axis=2)).astype(bf16)

    # W0T [128, NTAPS*COUT] bf16: lhsT per (branch, tap) with the 1x1 fuse conv
    # folded in (W''_k = W_k @ Wf_br^T), device layout
    WFT = [wf[:, :COUT, 0, 0].T, wf[:, COUT:, 0, 0].T]    # [84in, 84out] per br
    W0T = np.zeros((NBR * KK, C, COUT), np.float32)
    for br, w in ((0, w0), (1, w1)):
        for k in range(KK):
            W0T[br * KK + k] = w[:, :, k // K, k % K].T @ WFT[br]
    W0T = np.ascontiguousarray(np.transpose(W0T, (1, 0, 2))).astype(bf16)

    BF = bfv.reshape(COUT, 1).astype(np.float32)
    return xPPp, IDX, WS, W0T, BF


def _build_nc():
    nc = bacc.Bacc()
    xpp_d = nc.declare_dram_parameter("xpp", [NSLOT + 2, 2 * C], bft, isOutput=False)
    idx_d = nc.declare_dram_parameter("idx", [P, NBR * KK * IDXCOLS], i16, isOutput=False)
    ws_d = nc.declare_dram_parameter("ws", [P, NBR * KK * WSCOLS], bft, isOutput=False)
    w0_d = nc.declare_dram_parameter("w0t", [C, NBR * KK * COUT], bft, isOutput=False)
    bf_d = nc.declare_dram_parameter("bfv", [COUT, 1], f32, isOutput=False)
    out_d = nc.declare_dram_parameter("out", [COUT, HW], bft, isOutput=True)

    # patch gather source: elem i = slot i (256 elems), read 512 elems (2 slots)
    src_ap = bass.AP(tensor=xpp_d, offset=0, ap=[[2 * C, NSLOT], [1, 4 * C]])

    copy_engs = [int(t) for t in COPY_ENGS.split(",")]

    with TileContext(nc) as tc:
        with tc.tile_pool(name="const", bufs=1) as const, \
             tc.tile_pool(name="gp", bufs=GP_BUFS) as gp, \
             tc.tile_pool(name="dgp", bufs=DIAG_BUFS) as dgp, \
             tc.tile_pool(name="sampp", bufs=SAMP_BUFS) as sampp, \
             tc.tile_pool(name="op", bufs=2) as op, \
             tc.tile_pool(name="tpp", bufs=TPP_BUFS, space="PSUM") as tpp, \
             tc.tile_pool(name="bigp", bufs=BIGP_BUFS, space="PSUM") as bigp:
            ident = const.tile([P, P], bft)
            make_identity(nc, ident[:])
            TIC = NBR * KK * IDXCOLS
            TWC = NBR * KK * WSCOLS
            # split the idx/ws loads so the first gathers launch ASAP
            NIH = 2 * (CHUNKS[0] // 16)
            NWH = 2 * ((CHUNKS[0] // 128) * 4)
            idx_t = const.tile([P, TIC], i16)
            nc.sync.dma_start(out=idx_t[:, 0:NIH], in_=idx_d[:, 0:NIH])
            ws_b = const.tile([P, TWC], bft)
            nc.sync.dma_start(out=ws_b[:, 0:NWH], in_=ws_d[:, 0:NWH])
            ws_t = const.tile([P, TWC], f32)
            nc.vector.tensor_copy(out=ws_t[:, 0:NWH], in_=ws_b[:, 0:NWH])
            nc.sync.dma_start(out=idx_t[:, NIH:], in_=idx_d[:, NIH:TIC])
            nc.sync.dma_start(out=ws_b[:, NWH:], in_=ws_d[:, NWH:TWC])
            nc.vector.tensor_copy(out=ws_t[:, NWH:], in_=ws_b[:, NWH:])
            w0_t = const.tile([C, NBR * KK, COUT], bft)
            nc.sync.dma_start(out=w0_t[:], in_=w0_d[:])
            bf_t = const.tile([COUT, 1], f32)
            nc.sync.dma_start(out=bf_t[:], in_=bf_d[:])

            # prebuild the final list's diag tiles during the idle startup
            # window (they depend only on ws), so the drain isn't gated on
            # the loaded diag-build pipeline
            nlast = (CHUNKS[-1] // 128) * 4 * PREBUILD_LISTS
            dlast = const.tile([P, nlast, P], bft)
            wo_last = TWC - nlast
            for di in range(nlast):
                nc.vector.tensor_scalar(
                    out=dlast[:, di, :], in0=ident[:],
                    scalar1=ws_t[:, wo_last + di:wo_last + di + 1],
                    scalar2=None, op0=mybir.AluOpType.mult,
                )

            ndiag = 0
            ncopy = 0
            io, wo = 0, 0
            dgrp = None
            for ci in range(NCH):
                off, sz = CH_OFF[ci], CHUNKS[ci]
                nb = sz // 128
                out_ps = bigp.tile([COUT, CHUNKS[0]], f32, tag="big")
                for br in range(NBR):
                    for k in range(KK):
                        t = br * KK + k
                        last = (ci == NCH - 1 and br == NBR - 1
                                and k == KK - 1 and sz >= 1024)
                        if last:
                            # split the final gather into separate tiles so
                            # the first half processes while the second half
                            # transfers (deps are tile-granular)
                            hsz = sz // 2
                            ghalves = []
                            for gh in range(2):
                                gt = gp.tile([P, CHUNKS[0] // 256, 4 * C], bft,
                                             tag="glast")
                                nc.gpsimd.dma_gather(
                                    out_ap=gt[:, 0:hsz // 128, :],
                                    in_ap=src_ap,
                                    idxs_ap=idx_t[:, io + gh * (hsz // 16):
                                                  io + (gh + 1) * (hsz // 16)],
                                    num_idxs=hsz, num_idxs_reg=hsz,
                                    elem_size=4 * C, elem_step=2 * C,
                                    transpose=False, single_packet=False,
                                )
                                ghalves.append(gt)
                            g = None
                        else:
                            g = gp.tile([P, CHUNKS[0] // 128, 4 * C], bft,
                                        tag="g")
                            nc.gpsimd.dma_gather(
                                out_ap=g[:, 0:nb, :], in_ap=src_ap,
                                idxs_ap=idx_t[:, io:io + sz // 16],
                                num_idxs=sz, num_idxs_reg=sz,
                                elem_size=4 * C, elem_step=2 * C, transpose=False,
                                single_packet=False,
                            )
                        sampT = sampp.tile([C, CHUNKS[0]], bft, tag="sampT")
                        stripb = 4 if last else 8
                        for qh in range((nb + stripb - 1) // stripb):
                            qnb = min(stripb, nb - qh * stripb)
                            tp = tpp.tile([C, 1024], f32, tag="tp")
                            for jb in range(qnb):
                                b = qh * stripb + jb
                                for c4 in range(4):
                                    if (ci == NCH - 1 and br == NBR - 1
                                            and k >= KK - PREBUILD_LISTS):
                                        di = ((k - (KK - PREBUILD_LISTS))
                                              * (CHUNKS[-1] // 128) * 4
                                              + b * 4 + c4)
                                        diag = dlast[:, di, :]
                                    else:
                                        gi = ndiag % DIAG_GRP
                                        if gi == 0:
                                            dgrp = dgp.tile([P, DIAG_GRP, P],
                                                            bft, tag="diag")
                                        diag = dgrp[:, gi, :]
                                        sc = ws_t[:, wo + b * 4 + c4:
                                                  wo + b * 4 + c4 + 1]
                                        on_act = (ndiag * DIAG_ACT_NUM) \
                                            % DIAG_ACT_DEN < DIAG_ACT_NUM
                                        ndiag += 1
                                        if on_act:
                                            nc.scalar.activation(
                                                out=diag, in_=ident[:],
                                                func=mybir.ActivationFunctionType.Identity,
                                                scale=sc,
                                            )
                                        else:
                                            nc.vector.tensor_scalar(
                                                out=diag, in0=ident[:],
                                                scalar1=sc, scalar2=None,
                                                op0=mybir.AluOpType.mult,
                                            )
                                    if last:
                                        hb = sz // 256
                                        gsrc = ghalves[b // hb][:, b % hb, :]
                                    else:
                                        gsrc = g[:, b, :]
                                    nc.tensor.matmul(
                                        out=tp[:, jb * P:(jb + 1) * P],
                                        lhsT=gsrc[:, c4 * C:(c4 + 1) * C],
                                        rhs=diag,
                                        start=(c4 == 0), stop=(c4 == 3),
                                    )
                            if ci == NCH - 1 and br == NBR - 1 and k >= KK - 1:
                                # drain region: DVE is idle, ACT is the
                                # chokepoint — move the tail evacs to DVE
                                ce = 1
                            else:
                                ce = copy_engs[ncopy % len(copy_engs)]
                            ncopy += 1
                            qoff = qh * stripb * P
                            dst = sampT[:, qoff:qoff + qnb * P]
                            if ce == 1:
                                nc.vector.tensor_copy(out=dst,
                                                      in_=tp[:, 0:qnb * P])
                            else:
                                nc.scalar.copy(out=dst, in_=tp[:, 0:qnb * P])
                            if last:
                                # deform per 512-strip so the drain pipelines
                                nc.tensor.matmul(
                                    out=out_ps[:, qoff:qoff + qnb * P],
                                    lhsT=w0_t[:, t, :],
                                    rhs=sampT[:, qoff:qoff + qnb * P],
                                    start=False, stop=True,
                                )
                                if qh == sz // 1024 - 1:
                                    # first half fully accumulated: drain it
                                    # while the remaining strips process
                                    out_sb = op.tile([COUT, CHUNKS[0]], bft,
                                                     tag="outsb")
                                    nc.scalar.activation(
                                        out=out_sb[:, 0:sz // 2],
                                        in_=out_ps[:, 0:sz // 2],
                                        func=mybir.ActivationFunctionType.Identity,
                                        bias=bf_t[:], scale=1.0,
                                    )
                                    nc.sync.dma_start(
                                        out=out_d[:, off:off + sz // 2],
                                        in_=out_sb[:, 0:sz // 2])
                        if not last:
                            for cs in range(0, sz, 512):
                                ce_ = min(cs + 512, sz)
                                nc.tensor.matmul(
                                    out=out_ps[:, cs:ce_],
                                    lhsT=w0_t[:, t, :],
                                    rhs=sampT[:, cs:ce_],
                                    start=(br == 0 and k == 0),
                                    stop=(br == NBR - 1 and k == KK - 1),
                                )
                        io += sz // 16
                        wo += (sz // 128) * 4
                out_sb = op.tile([COUT, CHUNKS[0]], bft, tag="outsb")
                if ci == NCH - 1 and sz >= 1024:
                    # second half only: the first half drained mid-strip-loop
                    nc.scalar.activation(
                        out=out_sb[:, sz // 2:sz], in_=out_ps[:, sz // 2:sz],
                        func=mybir.ActivationFunctionType.Identity,
                        bias=bf_t[:], scale=1.0,
                    )
                    nc.sync.dma_start(out=out_d[:, off + sz // 2:off + sz],
                                      in_=out_sb[:, sz // 2:sz])
                else:
                    nc.scalar.activation(
                        out=out_sb[:, 0:sz], in_=out_ps[:, 0:sz],
                        func=mybir.ActivationFunctionType.Identity, bias=bf_t[:],
                        scale=1.0,
                    )
                    nc.sync.dma_start(out=out_d[:, off:off + sz],
                                      in_=out_sb[:, 0:sz])
    nc.finalize()
    return nc


def kernel(x, dm0, dm1, w0, w1, wf, bf):
    x = np.asarray(x, np.float32)
    dm0 = np.asarray(dm0, np.float32)
    dm1 = np.asarray(dm1, np.float32)
    w0 = np.asarray(w0, np.float32)
    w1 = np.asarray(w1, np.float32)
    wf = np.asarray(wf, np.float32)
    bfv = np.asarray(bf, np.float32)

    xPPp, IDX, WS, W0T, BF = _host_precompute(x, dm0, dm1, w0, w1, wf, bfv)

    if "nc" not in _CACHE:
        _CACHE["nc"] = _build_nc()
    nc = _CACHE["nc"]

    in_maps = [
        {
            "xpp": np.ascontiguousarray(xPPp[i]),
            "idx": np.ascontiguousarray(IDX[i].reshape(P, -1)),
            "ws": np.ascontiguousarray(WS[i].reshape(P, -1)),
            "w0t": W0T.reshape(C, -1),
            "bfv": BF,
        }
        for i in range(B)
    ]
    res = run_bass_kernel_spmd(nc, in_maps, core_ids=list(range(B)),
                               **_CACHE.get("run_kwargs", {}))
    _CACHE["last_results"] = res
    out = np.stack([np.asarray(res.results[i]["out"], np.float32)
                    for i in range(B)])
    return out.reshape(B, COUT, H, W)
